# revision 1
# baseline (speedup 1.0000x reference)
"""AtomDecoderLayer (GNN message passing) on 8 trn2 NeuronCores via Bass/Tile.

Sharding: flattened (batch, node) rows 2*1024 = 2048 -> 8 shards of 256 nodes
(8192 edge-rows each). Weights replicated. The edge_index gather runs on
device: every core computes the full node_msg table (2048, 256) -> DRAM
scratch, then indirect-DMA-gathers its 8192 rows (one batched xbar transpose
per chunk moves them into feature-major layout).

Layout: activations are feature-on-partition ("T space"); every dense matmul
is lhsT=weight (K,M<=128 slices), rhs=activation^T, accumulating K tiles in
PSUM. bf16 matmul inputs, fp32 accumulation. LayerNorm gain/bias are folded
into the following weight on the host; LN stats come from ones-lhsT matmuls
(rows broadcast across partitions); rstd = exp(-0.5 ln(var)) keeps the whole
chunk on two ACT tables. The target-node projection is injected into the
msg0 PSUM by an extra matmul against a constant node->edge indicator. The
GAT attention einsum uses a block-diagonal rhs trick: per 128-edge-row tile
(4 nodes x 32 neighbors), lhsT = msg rows (row space), rhs = A where
A[:, 17g:17g+16] = exp(ab) of node g and A[:, 17g+16] = edge mask; the
matmul yields both the unnormalized attention-weighted message sums (m) and
the masked mean-pool sums, and a ones-lhsT matmul over A yields softmax +
pool denominators. og is a small per-head projection of m (replaces the big
msg @ W_gat_value matmul). The attention tail of chunk c is pipelined into
iteration c+1 so ACT table switches stay at two per chunk.

Host does layout transforms / casts / index arithmetic only; all FLOPs run
on device. Self-contained: hardcodes b=2, n=1024, k=32, dim=512, pdim=256,
msg=256, H=16, D=64.
"""

import numpy as np

B, N, K = 2, 1024, 32
DIM, PD, MD = 512, 256, 256
H, DH = 16, 64
NC = 8
R = B * N // NC            # 256 nodes per core
RK = R * K                 # 8192 edge rows per core
NT = RK // 128             # 64 row-tiles per core
NCH = 8                    # chunks per core
CT = NT // NCH             # 8 row-tiles per chunk
CR = CT * 128              # 1024 edge rows per chunk
CNODE = CR // K            # 32 nodes per chunk
NS = CR // 512             # 2 512-col subtiles per chunk
LN_EPS = 1e-5
NEG = 0.01
MASK_OFF = 60.0

_CACHE = {}


# =========================================================================
# device kernel (Bass IR)
# =========================================================================
def _build_nc(debug=False):
    import concourse.bacc as bacc
    import concourse.bass as bass
    import concourse.mybir as mybir
    import concourse.tile as tile

    dt = mybir.dt
    AF = mybir.ActivationFunctionType
    OP = mybir.AluOpType
    f32, bf16 = dt.float32, dt.bfloat16

    nc = bacc.Bacc(None, target_bir_lowering=False)

    def din(name, shape, d=bf16):
        return nc.dram_tensor(name, shape, d, kind="ExternalInput")

    # per-core inputs
    edge_t = din("edge_t", [PD, RK])                 # edge_repr^T (bf16)
    gidx = din("gidx", [128, NT], dt.int32)          # gather row ids [p, t]
    gidx16 = din("gidx16", [128, NCH * (CR // 16)], dt.int16)
    emo = din("emo", [128, NT], f32)                 # (em-1)*MASK_OFF [p, t]
    emb = din("emb", [128, NT])                      # em (bf16) [p, t]
    node_sh_t = din("node_sh_t", [128, 4 * R])       # node shard^T bf16
    node_sh_f = din("node_sh_f", [128, 4 * R], f32)
    # replicated inputs
    node_t = din("node_t", [128, 4 * B * N])         # full node_repr^T bf16
    w_src_r = din("w_src_r", [128, 4 * MD])          # rhs layout
    w_edge_l = din("w_edge_l", [128, 2 * MD])
    w_tgt_r = din("w_tgt_r", [128, 4 * MD])          # rhs layout
    w1_l = din("w1_l", [128, 2 * 4 * MD])
    w2_l = din("w2_l", [128, 8 * MD])
    wmsg_l = din("wmsg_l", [128, 2 * MD])
    wab_l = din("wab_l", [128, 2 * H])
    wv_l = din("wv_l", [128, 2 * H * DH])
    wgate_l = din("wgate_l", [128, 4 * MD])
    wggate_l = din("wggate_l", [128, 4 * H * DH])
    wout_l = din("wout_l", [128, 2 * DIM])
    wgo_l = din("wgo_l", [128, 8 * DIM])
    w1n_l = din("w1n_l", [128, 4 * 4 * DIM])
    w2n_l = din("w2n_l", [128, 16 * DIM])
    sel_l = din("sel_l", [H, 8 * 128])               # head-broadcast indicator
    ebk_l = din("ebk_l", [CNODE, CR])                # node->edge indicator
    biases = din("biases", [128, 40], f32)           # packed bias columns

    out_t = nc.dram_tensor("out_t", [DIM, R], f32, kind="ExternalOutput")

    dbg = {}
    if debug:
        for nm, shape, d in [
            ("d_gat", [128, 2 * CR], bf16), ("d_m0b", [128, 2 * CR], bf16),
            ("d_xln", [128, 2 * CR], bf16), ("d_msg", [128, 2 * CR], bf16),
            ("d_abr", [128, CT * H], bf16), ("d_abm", [128, CT * H], bf16),
            ("d_A", [128, CT * 68], bf16), ("d_mrow", [128, 2 * CR], bf16),
            ("d_mT", [128, 2 * NT * 68], bf16), ("d_den", [1, NT * 68], f32),
            ("d_s16", [H, R], f32), ("d_g1", [128, 2 * R], f32),
            ("d_o", [128, 2 * R], bf16), ("d_og", [128, 8 * R], bf16),
            ("d_x1f", [128, 4 * R], f32), ("d_tgt", [CNODE, NCH * 2 * 128], bf16),
        ]:
            dbg[nm] = nc.dram_tensor(nm, shape, d, kind="ExternalOutput")

    # packed bias column offsets
    B1T, B2M, BGA, BGG, B1NT, B2N = 0, 8, 10, 12, 20, 36

    with tile.TileContext(nc) as tc:
        with (
            tc.tile_pool(name="per", bufs=1) as per,
            tc.tile_pool(name="wts", bufs=1) as wts,
            tc.tile_pool(name="dram", bufs=1, space="DRAM") as dpool,
        ):
            def load(t, shape, d=bf16, pool=wts):
                s = pool.tile(shape, d, tag=t.name)
                nc.sync.dma_start(s[:], t[:])
                return s

            node_sh_s = load(node_sh_t, [128, 4 * R], pool=per)
            node_shf_s = load(node_sh_f, [128, 4 * R], f32, pool=per)
            gidx_s = load(gidx, [128, NT], dt.int32, pool=per)
            gidx16_s = load(gidx16, [128, NCH * (CR // 16)], dt.int16,
                            pool=per)
            emo_s = load(emo, [128, NT], f32, pool=per)
            emb_s = load(emb, [128, NT], pool=per)
            bias_s = load(biases, [128, 40], f32, pool=per)

            def bia(base, m):
                return bias_s[:, base + m:base + m + 1]

            def rsqrt_dve(pool, ap, tag):
                """In-place y = 1/sqrt(ap) on DVE (bit trick + 1 Newton)."""
                shp = list(ap.shape)
                y0 = pool.tile(shp, f32, tag=tag + "_y0")
                t1 = pool.tile(shp, f32, tag=tag + "_t1")
                vi = ap.bitcast(dt.int32)
                y0i = y0[:].bitcast(dt.int32)
                nc.vector.tensor_scalar(
                    out=y0i, in0=vi, scalar1=1, scalar2=None,
                    op0=OP.arith_shift_right)
                nc.vector.tensor_scalar(
                    out=y0i, in0=y0i, scalar1=-1, scalar2=0x5f3759df,
                    op0=OP.mult, op1=OP.add)
                # t1 = 1.5 - 0.5*v*y0*y0 ; y = y0*t1
                nc.vector.tensor_tensor(out=t1[:], in0=ap, in1=y0[:],
                                        op=OP.mult)
                nc.vector.tensor_tensor(out=t1[:], in0=t1[:], in1=y0[:],
                                        op=OP.mult)
                nc.vector.tensor_scalar(
                    out=t1[:], in0=t1[:], scalar1=-0.5, scalar2=1.5,
                    op0=OP.mult, op1=OP.add)
                nc.vector.tensor_tensor(out=ap, in0=y0[:], in1=t1[:],
                                        op=OP.mult)

            ones_s = per.tile([128, 128], bf16, tag="ones")
            nc.vector.memset(ones_s[:], 1.0)

            g1 = per.tile([128, 2 * R], f32, tag="g1")
            g2 = per.tile([128, 8 * R], f32, tag="g2")

            table = dpool.tile([B * N, MD], bf16, tag="table")

            # persistent activations
            tgt32 = per.tile([CNODE, NCH * 2 * 128], bf16, tag="tgt32")
            mT_s = per.tile([128, 2 * NT * 68], bf16, tag="mT")   # einsum out
            den_s = per.tile([1, NT * 68], f32, tag="den")        # denominators
            den_v = den_s[:1, :].rearrange("o (c T g) -> o c T g", c=17, g=4)

            # ---------------- phase 0: node_msg table + tgt --------------
            with (
                tc.tile_pool(name="p0s", bufs=2) as p0s,
                tc.tile_pool(name="p0p", bufs=4, space="PSUM") as p0p,
            ):
                node_t_s = load(node_t, [128, 4 * B * N], pool=p0s)
                w_src_s = load(w_src_r, [128, 4 * MD], pool=p0s)
                w_tgt_s = load(w_tgt_r, [128, 4 * MD], pool=p0s)
                wgate_s = load(wgate_l, [128, 4 * MD], pool=p0s)
                wggate_s = load(wggate_l, [128, 4 * H * DH], pool=p0s)
                for s in range(B * N // 128):
                    p = p0p.tile([128, MD], f32, tag="pt")
                    c0 = s * 128
                    for k in range(4):
                        nc.tensor.matmul(
                            p[:],
                            node_t_s[:, k * B * N + c0:k * B * N + c0 + 128],
                            w_src_s[:, k * MD:(k + 1) * MD],
                            start=(k == 0), stop=(k == 3))
                    sb = p0s.tile([128, MD], bf16, tag="tbev")
                    nc.vector.tensor_copy(sb[:], p[:])
                    nc.sync.dma_start(table[c0:c0 + 128, :], sb[:])

                # tgt in row space, 32 nodes per chunk at partitions 0-31
                for c in range(NCH):
                    p = p0p.tile([CNODE, MD], f32, tag="pg")
                    for k in range(4):
                        nc.tensor.matmul(
                            p[:],
                            node_sh_s[:, k * R + c * CNODE:
                                      k * R + c * CNODE + CNODE],
                            w_tgt_s[:, k * MD:(k + 1) * MD],
                            start=(k == 0), stop=(k == 3))
                    nc.vector.tensor_copy(
                        tgt32[:, :].rearrange("p (c j e) -> p c (j e)", j=2,
                                              c=NCH)[:, c, :], p[:])
                if debug:
                    nc.sync.dma_start(dbg["d_tgt"][:], tgt32[:])

                # gates (raw; pool-denominator folded in at the tail)
                for m in range(2):
                    p = p0p.tile([128, R], f32, tag="pt")
                    for k in range(4):
                        nc.tensor.matmul(
                            p[:],
                            wgate_s[:, k * MD + m * 128:k * MD + m * 128 + 128],
                            node_sh_s[:, k * R:(k + 1) * R],
                            start=(k == 0), stop=(k == 3))
                    nc.scalar.activation(g1[:, m * R:(m + 1) * R], p[:],
                                         AF.Sigmoid, bias=bia(BGA, m))
                for m in range(8):
                    p = p0p.tile([128, R], f32, tag="pt")
                    for k in range(4):
                        nc.tensor.matmul(
                            p[:],
                            wggate_s[:, k * H * DH + m * 128:
                                     k * H * DH + m * 128 + 128],
                            node_sh_s[:, k * R:(k + 1) * R],
                            start=(k == 0), stop=(k == 3))
                    nc.scalar.activation(g2[:, m * R:(m + 1) * R], p[:],
                                         AF.Sigmoid, bias=bia(BGG, m))

            w_edge_s = load(w_edge_l, [128, 2 * MD])
            w1_s = load(w1_l, [128, 2 * 4 * MD])
            w2_s = load(w2_l, [128, 8 * MD])
            wmsg_s = load(wmsg_l, [128, 2 * MD])
            wab_s = load(wab_l, [128, 2 * H])
            ebk_s = load(ebk_l, [CNODE, CR])
            wv_s = load(wv_l, [128, 2 * H * DH])
            sel_s = load(sel_l, [H, 8 * 128])
            wout_s = load(wout_l, [128, 2 * DIM])
            wgo_s = load(wgo_l, [128, 8 * DIM])

            # ---------------- chunk loop (attention tail pipelined) ------
            with (
                tc.tile_pool(name="ch1", bufs=1) as ch1,
                tc.tile_pool(name="ch2", bufs=2) as ch2,
                tc.tile_pool(name="ch3", bufs=3) as ch3,
                tc.tile_pool(name="cpw", bufs=2, space="PSUM") as cpw,
                tc.tile_pool(name="cpm", bufs=2, space="PSUM") as cpm,
                tc.tile_pool(name="cpe", bufs=2, space="PSUM") as cpe,
            ):
                mT4 = mT_s[:, :].rearrange("p (j t c) -> p j t c", j=2, t=NT)

                def front(c):
                    c0 = c * CR
                    ech = ch2.tile([128, 2 * CR], bf16, tag="ech")
                    for k in range(2):
                        nc.sync.dma_start(
                            ech[:, k * CR:(k + 1) * CR],
                            edge_t[k * 128:(k + 1) * 128, c0:c0 + CR])

                    # gather (row space, one indirect DMA), xbar to T space
                    grow = ch3.tile([128, CT * MD], bf16, tag="grow")
                    for t in range(CT):
                        nc.gpsimd.indirect_dma_start(
                            out=grow[:, t * MD:(t + 1) * MD], out_offset=None,
                            in_=table[:],
                            in_offset=bass.IndirectOffsetOnAxis(
                                ap=gidx_s[:, c * CT + t:c * CT + t + 1],
                                axis=0))
                    gat = ch2.tile([128, 2 * CR], bf16, tag="gat")
                    nc.sync.dma_start(
                        gat[:, :].rearrange("p (t j e) -> p (t j) e", j=2,
                                            e=128),
                        grow[:], transpose=True)
                    gat4 = gat[:, :].rearrange("p (t j e) -> p t j e",
                                               j=2, t=CT)
                    if debug and c == 0:
                        nc.sync.dma_start(dbg["d_gat"][:], gat[:])

                    # msg0 = edge@We + tgt-indicator-mm (PSUM) + gath (DVE)
                    m0b = ch2.tile([128, 2 * CR], bf16, tag="m0b")
                    for m in range(2):
                        for s in range(NS):
                            p = cpm.tile([128, 512], f32, tag="pm")
                            for k in range(2):
                                nc.tensor.matmul(
                                    p[:],
                                    w_edge_s[:, k * MD + m * 128:
                                             k * MD + m * 128 + 128],
                                    ech[:, k * CR + s * 512:
                                        k * CR + s * 512 + 512],
                                    start=(k == 0), stop=False)
                            nc.tensor.matmul(
                                p[:],
                                tgt32[:, (c * 2 + m) * 128:
                                      (c * 2 + m) * 128 + 128],
                                ebk_s[:, s * 512:s * 512 + 512],
                                start=False, stop=True)
                            p3t = p[:].rearrange("p (t e) -> p t e", e=128)
                            m3 = m0b[:, m * CR + s * 512:m * CR + s * 512
                                     + 512].rearrange("p (t e) -> p t e",
                                                      e=128)
                            nc.vector.tensor_tensor(
                                out=m3, in0=p3t,
                                in1=gat4[:, 4 * s:4 * s + 4, m, :],
                                op=OP.add)
                    if debug and c == 0:
                        nc.sync.dma_start(dbg["d_m0b"][:], m0b[:])

                    # LN stats
                    sq = ch1.tile([128, 2 * CR], bf16, tag="sq")
                    for m in range(2):
                        nc.vector.tensor_tensor(
                            out=sq[:, m * CR:(m + 1) * CR],
                            in0=m0b[:, m * CR:(m + 1) * CR],
                            in1=m0b[:, m * CR:(m + 1) * CR], op=OP.mult)
                    mu = ch2.tile([128, CR], f32, tag="mu")
                    st = ch2.tile([128, CR], f32, tag="st")
                    for src, dst, bb in ((m0b, mu, 0.0), (sq, st, LN_EPS)):
                        for s in range(NS):
                            p = cpm.tile([128, 512], f32, tag="pms")
                            for k in range(2):
                                nc.tensor.matmul(
                                    p[:], ones_s[:],
                                    src[:, k * CR + s * 512:
                                        k * CR + s * 512 + 512],
                                    start=(k == 0), stop=(k == 1))
                            nc.scalar.activation(
                                dst[:, s * 512:(s + 1) * 512], p[:],
                                AF.Copy, bias=float(bb), scale=1.0 / MD)
                    mu2 = ch1.tile([128, CR], f32, tag="mu2")
                    nc.vector.tensor_tensor(out=mu2[:], in0=mu[:], in1=mu[:],
                                            op=OP.mult)
                    nc.vector.tensor_tensor(out=st[:], in0=st[:], in1=mu2[:],
                                            op=OP.subtract)
                    rsqrt_dve(ch1, st[:], "rs")
                    xln = ch2.tile([128, 2 * CR], bf16, tag="xln")
                    for m in range(2):
                        sl = slice(m * CR, (m + 1) * CR)
                        nc.vector.tensor_tensor(out=xln[:, sl], in0=m0b[:, sl],
                                                in1=mu[:], op=OP.subtract)
                        nc.vector.tensor_tensor(out=xln[:, sl], in0=xln[:, sl],
                                                in1=st[:], op=OP.mult)
                    if debug and c == 0:
                        nc.sync.dma_start(dbg["d_xln"][:], xln[:])

                    # msg MLP
                    msg = ch2.tile([128, 2 * CR], bf16, tag="msg")
                    for s in range(NS):
                        h1 = ch2.tile([128, 8 * 512], bf16, tag="h1")
                        for m in range(8):
                            p = cpw.tile([128, 512], f32, tag="pw")
                            for k in range(2):
                                nc.tensor.matmul(
                                    p[:],
                                    w1_s[:, k * 4 * MD + m * 128:
                                         k * 4 * MD + m * 128 + 128],
                                    xln[:, k * CR + s * 512:
                                        k * CR + s * 512 + 512],
                                    start=(k == 0), stop=(k == 1))
                            nc.scalar.activation(
                                h1[:, m * 512:(m + 1) * 512], p[:], AF.Gelu,
                                bias=bia(B1T, m))
                        for m2 in range(2):
                            p = cpw.tile([128, 512], f32, tag="pw")
                            for k in range(8):
                                nc.tensor.matmul(
                                    p[:],
                                    w2_s[:, k * MD + m2 * 128:
                                         k * MD + m2 * 128 + 128],
                                    h1[:, k * 512:(k + 1) * 512],
                                    start=(k == 0), stop=(k == 7))
                            nc.scalar.activation(
                                msg[:, m2 * CR + s * 512:
                                    m2 * CR + s * 512 + 512],
                                p[:], AF.Identity, bias=bia(B2M, m2))
                    if debug and c == 0:
                        nc.sync.dma_start(dbg["d_msg"][:], msg[:])

                    # z = leaky(msg @ Wmsg); ab = z @ Wab
                    zt = ch1.tile([128, 2 * CR], bf16, tag="zt")
                    for m in range(2):
                        for s in range(NS):
                            p = cpw.tile([128, 512], f32, tag="pw")
                            for k in range(2):
                                nc.tensor.matmul(
                                    p[:],
                                    wmsg_s[:, k * MD + m * 128:
                                           k * MD + m * 128 + 128],
                                    msg[:, k * CR + s * 512:
                                        k * CR + s * 512 + 512],
                                    start=(k == 0), stop=(k == 1))
                            nc.scalar.activation(
                                zt[:, m * CR + s * 512:m * CR + s * 512 + 512],
                                p[:], AF.Lrelu, alpha=NEG)
                    abt = ch1.tile([H, CR], bf16, tag="abt")
                    for s in range(NS):
                        pfull = cpe.tile([128, 512], f32, tag="pback")
                        p = pfull[:H, :]
                        for k in range(2):
                            nc.tensor.matmul(
                                p[:], wab_s[:, k * H:(k + 1) * H],
                                zt[:, k * CR + s * 512:k * CR + s * 512 + 512],
                                start=(k == 0), stop=(k == 1))
                        nc.vector.tensor_copy(abt[:, s * 512:(s + 1) * 512],
                                              p[:])
                    abr = ch2.tile([128, CT * H], bf16, tag="abr")
                    nc.sync.dma_start(
                        abr[:, :].rearrange("p (t h) -> p t h", h=H),
                        abt[:], transpose=True)
                    if debug and c == 0:
                        nc.sync.dma_start(dbg["d_abr"][:], abr[:])

                    # msg -> row space (one xbar transpose)
                    mrow = ch2.tile([128, 2 * CR], bf16, tag="mrow")
                    nc.sync.dma_start(
                        mrow[:, :].rearrange("p (q e) -> p q e", e=128),
                        msg[:], transpose=True)
                    if debug and c == 0:
                        nc.sync.dma_start(dbg["d_mrow"][:], mrow[:])
                    abr3 = abr[:, :].rearrange("p (t h) -> p t h", h=H)
                    mrow4 = mrow[:, :].rearrange("p (j t e) -> p j t e",
                                                 j=2, t=CT)
                    # exp(ab + emo) on DVE: 2^y, y = ab*log2e + emo2
                    W = CT * H
                    ey = ch1.tile([128, W], f32, tag="ey")
                    ez = ch1.tile([128, W], f32, tag="ez")
                    ef = ch1.tile([128, W], f32, tag="ef")
                    ep = ch1.tile([128, W], f32, tag="ep")
                    nc.vector.tensor_scalar_mul(ey[:], abr[:], 1.4426950408889634)
                    e3 = emo_s[:, c * CT:(c + 1) * CT].to_broadcast(
                        [128, CT, H])
                    nc.vector.tensor_tensor(
                        out=ey[:].rearrange("p (t h) -> p t h", h=H),
                        in0=ey[:].rearrange("p (t h) -> p t h", h=H),
                        in1=e3, op=OP.add)
                    MAGIC = 12582912.0
                    en = ch1.tile([128, W], f32, tag="en")
                    nc.vector.tensor_scalar_add(ez[:], ey[:], MAGIC)
                    nc.vector.tensor_scalar_sub(en[:], ez[:], MAGIC)
                    nc.vector.tensor_tensor(out=ef[:], in0=ey[:], in1=en[:],
                                            op=OP.subtract)
                    # p = 1 + f(c1 + f(c2 + f c3))
                    nc.vector.tensor_scalar(
                        out=ep[:], in0=ef[:], scalar1=0.0555041,
                        scalar2=0.240227, op0=OP.mult, op1=OP.add)
                    nc.vector.tensor_tensor(out=ep[:], in0=ep[:], in1=ef[:],
                                            op=OP.mult)
                    nc.vector.tensor_scalar_add(ep[:], ep[:], 0.6931472)
                    nc.vector.tensor_tensor(out=ep[:], in0=ep[:], in1=ef[:],
                                            op=OP.mult)
                    nc.vector.tensor_scalar_add(ep[:], ep[:], 1.0)
                    # 2^n: bit pattern (n+127)*2^23 built in float, cast
                    nc.vector.tensor_scalar(
                        out=ez[:], in0=en[:], scalar1=127.0,
                        scalar2=8388608.0, op0=OP.add, op1=OP.mult)
                    ei = ch1.tile([128, W], dt.int32, tag="ei")
                    nc.vector.tensor_copy(ei[:], ez[:])
                    abm = ch1.tile([128, CT * H], bf16, tag="abm")
                    abm3 = abm[:, :].rearrange("p (t h) -> p t h", h=H)
                    nc.vector.tensor_tensor(out=abm[:], in0=ep[:],
                                            in1=ei[:].bitcast(f32),
                                            op=OP.mult)
                    if debug and c == 0:
                        nc.sync.dma_start(dbg["d_abm"][:], abm[:])

                    A = ch1.tile([128, CT * 68], bf16, tag="A")
                    nc.vector.memset(A[:], 0.0)
                    A3 = A[:, :].rearrange("p (t c) -> p t c", c=68)
                    em3 = emb_s[:, c * CT:(c + 1) * CT].to_broadcast(
                        [128, CT, 1])
                    for g in range(4):
                        rs = slice(32 * g, 32 * g + 32)
                        nc.vector.tensor_copy(A3[rs, :, 17 * g:17 * g + 16],
                                              abm3[rs, :, :])
                        nc.vector.tensor_copy(
                            A3[rs, :, 17 * g + 16:17 * g + 17], em3[rs, :, :])
                    if debug and c == 0:
                        nc.sync.dma_start(dbg["d_A"][:], A[:])

                    for t in range(CT):
                        for m in range(2):
                            pfull = cpe.tile([128, 512], f32,
                                             tag="pback")
                            p = pfull[:, :68]
                            nc.tensor.matmul(
                                p[:], mrow4[:, m, t, :], A3[:, t, :],
                                start=True, stop=True)
                            nc.vector.tensor_copy(mT4[:, m, c * CT + t, :],
                                                  p[:])
                    for t0, nt_ in ((0, 7), (7, 1)):
                        w = nt_ * 68
                        p = cpe.tile([128, 512], f32, tag="pback")
                        nc.tensor.matmul(p[:, :w], ones_s[:],
                                         A[:, t0 * 68:t0 * 68 + w],
                                         start=True, stop=True)
                        nc.scalar.copy(
                            den_v[:, :, c * CT + t0:c * CT + t0 + nt_, :]
                            .rearrange("o c T g -> o T g c"),
                            p[:1, :w].rearrange("o (t g c) -> o t g c",
                                                g=4, c=17))

                for c in range(NCH):
                    front(c)

            # ---------------- tail ---------------------------------------
            with (
                tc.tile_pool(name="tl", bufs=1) as tl,
                tc.tile_pool(name="tp", bufs=4, space="PSUM") as tp,
                tc.tile_pool(name="tp2", bufs=2, space="PSUM") as tp2,
            ):
                w1n_s = load(w1n_l, [128, 4 * 4 * DIM], pool=tl)
                w2n_s = load(w2n_l, [128, 16 * DIM], pool=tl)
                if debug:
                    nc.sync.dma_start(dbg["d_mT"][:], mT_s[:])
                    nc.sync.dma_start(dbg["d_den"][:], den_s[:])
                # s16(h, node) = 1/softmax_den; sE(node) = 1/(pool_den+1e-6)
                s16 = tl.tile([H, R], f32, tag="s16")
                nc.sync.dma_start(s16[:], den_s[:1, 0:H * R])
                rsqrt_dve(tl, s16[:], "rs16")
                nc.vector.tensor_tensor(out=s16[:], in0=s16[:], in1=s16[:],
                                        op=OP.mult)
                if debug:
                    nc.sync.dma_start(dbg["d_s16"][:], s16[:])
                s16b = tl.tile([H, R], bf16, tag="s16b")
                nc.vector.tensor_copy(s16b[:], s16[:])
                sE = tl.tile([1, R], f32, tag="sE")
                nc.sync.dma_start(sE[:], den_s[:1, H * R:17 * R])
                nc.vector.tensor_scalar_add(sE[:], sE[:], 1e-6)
                rsqrt_dve(tl, sE[:], "rsE")
                nc.vector.tensor_tensor(out=sE[:], in0=sE[:], in1=sE[:],
                                        op=OP.mult)
                sEb = tl.tile([1, R], bf16, tag="sEb")
                nc.vector.tensor_copy(sEb[:], sE[:])
                sE_ps = tp2.tile([128, R], f32, tag="psE")
                nc.tensor.matmul(sE_ps[:], ones_s[:1, :], sEb[:],
                                 start=True, stop=True)

                # fold pool denominator into g1
                for m in range(2):
                    nc.vector.tensor_tensor(out=g1[:, m * R:(m + 1) * R],
                                            in0=g1[:, m * R:(m + 1) * R],
                                            in1=sE_ps[:], op=OP.mult)
                if debug:
                    nc.sync.dma_start(dbg["d_g1"][:], g1[:])

                mT5 = mT_s[:, :].rearrange("p (j t g c) -> p j t g c",
                                           j=2, t=NT, g=4)
                # o^T gated
                o_s = tl.tile([128, 2 * R], bf16, tag="o")
                for m in range(2):
                    nc.vector.tensor_tensor(
                        out=o_s[:, m * R:(m + 1) * R].rearrange(
                            "p (t g) -> p t g", g=4),
                        in0=mT5[:, m, :, :, 16],
                        in1=g1[:, m * R:(m + 1) * R].rearrange(
                            "p (t g) -> p t g", g=4),
                        op=OP.mult)
                if debug:
                    nc.sync.dma_start(dbg["d_o"][:], o_s[:])

                # og: per-head projection of m, then softmax scale and gate
                og_s = tl.tile([128, 8 * R], bf16, tag="og")
                for tpi in range(8):
                    p = tp.tile([128, R], f32, tag="pt")
                    for hh in range(2):
                        h = 2 * tpi + hh
                        for k in range(2):
                            nc.tensor.matmul(
                                p[64 * hh:64 * hh + 64, :],
                                wv_s[:, k * H * DH + h * DH:
                                     k * H * DH + h * DH + DH],
                                mT5[:, k, :, :, h].rearrange(
                                    "p t g -> p (t g)"),
                                start=(k == 0), stop=(k == 1))
                    sp = tp2.tile([128, R], f32, tag="ps16")
                    nc.tensor.matmul(sp[:], sel_s[:, tpi * 128:(tpi + 1) * 128],
                                     s16b[:], start=True, stop=True)
                    sg = tl.tile([128, R], f32, tag="sg")
                    nc.vector.tensor_tensor(out=sg[:], in0=sp[:],
                                            in1=g2[:, tpi * R:(tpi + 1) * R],
                                            op=OP.mult)
                    nc.vector.tensor_tensor(
                        out=og_s[:, tpi * R:(tpi + 1) * R], in0=p[:],
                        in1=sg[:], op=OP.mult)
                if debug:
                    nc.sync.dma_start(dbg["d_og"][:], og_s[:])

                # dh = W_out.T @ o + W_go.T @ og;  x1 = node + dh
                x1f = tl.tile([128, 4 * R], f32, tag="x1f")
                x1b = tl.tile([128, 4 * R], bf16, tag="x1b")
                for m in range(4):
                    p = tp.tile([128, R], f32, tag="pt")
                    for k in range(2):
                        nc.tensor.matmul(
                            p[:],
                            wout_s[:, k * DIM + m * 128:k * DIM + m * 128 + 128],
                            o_s[:, k * R:(k + 1) * R],
                            start=(k == 0), stop=False)
                    for k in range(8):
                        nc.tensor.matmul(
                            p[:],
                            wgo_s[:, k * DIM + m * 128:k * DIM + m * 128 + 128],
                            og_s[:, k * R:(k + 1) * R],
                            start=False, stop=(k == 7))
                    sl = slice(m * R, (m + 1) * R)
                    nc.vector.tensor_tensor(out=x1f[:, sl], in0=p[:],
                                            in1=node_shf_s[:, sl], op=OP.add)
                    nc.vector.tensor_copy(x1b[:, sl], x1f[:, sl])
                if debug:
                    nc.sync.dma_start(dbg["d_x1f"][:], x1f[:])

                # node MLP with LN folded into W1n
                sqn = tl.tile([128, 4 * R], bf16, tag="sqn")
                for m in range(4):
                    nc.vector.tensor_tensor(
                        out=sqn[:, m * R:(m + 1) * R],
                        in0=x1b[:, m * R:(m + 1) * R],
                        in1=x1b[:, m * R:(m + 1) * R], op=OP.mult)
                mun = tl.tile([128, R], f32, tag="mun")
                stn = tl.tile([128, R], f32, tag="stn")
                for src, dst, bb in ((x1b, mun, 0.0), (sqn, stn, LN_EPS)):
                    p = tp.tile([128, R], f32, tag="pt")
                    for k in range(4):
                        nc.tensor.matmul(p[:], ones_s[:],
                                         src[:, k * R:(k + 1) * R],
                                         start=(k == 0), stop=(k == 3))
                    nc.scalar.activation(dst[:], p[:], AF.Copy,
                                         bias=float(bb), scale=1.0 / DIM)
                mu2n = tl.tile([128, R], f32, tag="mu2n")
                nc.vector.tensor_tensor(out=mu2n[:], in0=mun[:], in1=mun[:],
                                        op=OP.mult)
                nc.vector.tensor_tensor(out=stn[:], in0=stn[:], in1=mu2n[:],
                                        op=OP.subtract)
                rsqrt_dve(tl, stn[:], "rsn")
                xlnn = tl.tile([128, 4 * R], bf16, tag="xlnn")
                for m in range(4):
                    sl = slice(m * R, (m + 1) * R)
                    nc.vector.tensor_tensor(out=xlnn[:, sl], in0=x1b[:, sl],
                                            in1=mun[:], op=OP.subtract)
                    nc.vector.tensor_tensor(out=xlnn[:, sl], in0=xlnn[:, sl],
                                            in1=stn[:], op=OP.mult)
                h2 = tl.tile([128, 16 * R], bf16, tag="h2")
                for m in range(16):
                    p = tp.tile([128, R], f32, tag="pt")
                    for k in range(4):
                        nc.tensor.matmul(
                            p[:],
                            w1n_s[:, k * 4 * DIM + m * 128:
                                  k * 4 * DIM + m * 128 + 128],
                            xlnn[:, k * R:(k + 1) * R],
                            start=(k == 0), stop=(k == 3))
                    nc.scalar.activation(h2[:, m * R:(m + 1) * R], p[:],
                                         AF.Gelu, bias=bia(B1NT, m))
                outs = tl.tile([128, 4 * R], f32, tag="outs")
                for m in range(4):
                    p = tp.tile([128, R], f32, tag="pt")
                    for k in range(16):
                        nc.tensor.matmul(
                            p[:],
                            w2n_s[:, k * DIM + m * 128:k * DIM + m * 128 + 128],
                            h2[:, k * R:(k + 1) * R],
                            start=(k == 0), stop=(k == 15))
                    sl = slice(m * R, (m + 1) * R)
                    nc.scalar.activation(p[:], p[:], AF.Identity,
                                         bias=bia(B2N, m))
                    nc.vector.tensor_tensor(out=outs[:, sl], in0=p[:],
                                            in1=x1f[:, sl], op=OP.add)
                    nc.sync.dma_start(out_t[m * 128:(m + 1) * 128, :],
                                      outs[:, sl])

    nc.compile()
    return nc


# =========================================================================
# host side
# =========================================================================
def _lhsT(w):
    """(K, M) f32 -> (128, kt*M) bf16 lhsT layout."""
    import ml_dtypes
    Kd, M = w.shape
    kt = Kd // 128
    return np.ascontiguousarray(
        w.reshape(kt, 128, M).transpose(1, 0, 2).reshape(128, kt * M)
    ).astype(ml_dtypes.bfloat16)


def _bias_cols(v):
    return np.ascontiguousarray(v.reshape(-1, 128).T).astype(np.float32)


def _prep(inputs):
    import ml_dtypes
    bf = ml_dtypes.bfloat16
    f32 = np.float32

    node = np.asarray(inputs['node_repr'], f32).reshape(B * N, DIM)
    edge = np.asarray(inputs['edge_repr'], f32).reshape(B * N * K, PD)
    eidx = np.asarray(inputs['edge_index']).reshape(B, N, K)
    emask = np.asarray(inputs['edge_mask'], f32).reshape(B * N, K)
    mask_bw = np.asarray(inputs['mask_bw'], f32)
    if not np.all(mask_bw == 1.0):
        return None

    g = np.asarray(inputs['msg_ln_g'], f32)
    b = np.asarray(inputs['msg_ln_b'], f32)
    W1 = np.asarray(inputs['msg_W1'], f32)
    W1p = g[:, None] * W1
    b1t = np.asarray(inputs['msg_b1'], f32) + b @ W1
    gn = np.asarray(inputs['node_ln_g'], f32)
    bn = np.asarray(inputs['node_ln_b'], f32)
    W1n = np.asarray(inputs['node_W1'], f32)
    W1np = gn[:, None] * W1n
    b1nt = np.asarray(inputs['node_b1'], f32) + bn @ W1n

    sel = np.zeros((H, 8 * 128), f32)
    for tp in range(8):
        for hh in range(2):
            sel[2 * tp + hh,
                tp * 128 + 64 * hh:tp * 128 + 64 * hh + 64] = 1.0

    # node->edge indicator: ebk[n, s*512 + e] = 1 iff n == s*16 + e//32
    ebk = np.zeros((CNODE, CR), f32)
    for r in range(CR):
        ebk[r // K, r] = 1.0

    biases = np.zeros((128, 40), f32)
    biases[:, 0:8] = _bias_cols(b1t)
    biases[:, 8:10] = _bias_cols(np.asarray(inputs['msg_b2'], f32))
    biases[:, 10:12] = _bias_cols(np.asarray(inputs['b_gate'], f32))
    biases[:, 12:20] = _bias_cols(np.asarray(inputs['b_gat_gate'], f32))
    biases[:, 20:36] = _bias_cols(b1nt)
    biases[:, 36:40] = _bias_cols(np.asarray(inputs['node_b2'], f32))

    def ktile(xt):
        Kd, n = xt.shape
        kt = Kd // 128
        return np.ascontiguousarray(
            xt.reshape(kt, 128, n).transpose(1, 0, 2).reshape(128, kt * n))

    def krhs(w):
        Kd, M = w.shape
        kt = Kd // 128
        return np.ascontiguousarray(
            w.reshape(kt, 128, M).transpose(1, 0, 2).reshape(128, kt * M))

    rep = {
        'node_t': ktile(node.T).astype(bf),
        'w_src_r': krhs(np.asarray(inputs['W_node_src'], f32)).astype(bf),
        'w_tgt_r': krhs(np.asarray(inputs['W_node_tgt'], f32)).astype(bf),
        'w_edge_l': _lhsT(np.asarray(inputs['W_edge_msg'], f32)),
        'w1_l': _lhsT(W1p),
        'w2_l': _lhsT(np.asarray(inputs['msg_W2'], f32)),
        'wmsg_l': _lhsT(np.asarray(inputs['W_msg'], f32)),
        'wab_l': _lhsT(np.asarray(inputs['W_attn_bias'], f32)),
        'wv_l': _lhsT(np.asarray(inputs['W_gat_value'], f32)),
        'wgate_l': _lhsT(np.asarray(inputs['W_gate'], f32)),
        'wggate_l': _lhsT(np.asarray(inputs['W_gat_gate'], f32)),
        'wout_l': _lhsT(np.asarray(inputs['W_out'], f32)),
        'wgo_l': _lhsT(np.asarray(inputs['W_gat_out'], f32)),
        'w1n_l': _lhsT(W1np),
        'w2n_l': _lhsT(np.asarray(inputs['node_W2'], f32)),
        'sel_l': sel.astype(bf),
        'ebk_l': ebk.astype(bf),
        'biases': biases,
    }

    boff = (np.arange(B)[:, None, None] * N).astype(np.int32)
    gidx_full = (eidx.astype(np.int32) + boff).reshape(B * N, K)

    in_maps = []
    for c in range(NC):
        rs = slice(c * R, (c + 1) * R)
        esh = edge[c * RK:(c + 1) * RK, :]
        em = emask[rs, :].reshape(RK)
        gi = gidx_full[rs, :].reshape(RK)
        m = dict(rep)
        m['edge_t'] = np.ascontiguousarray(esh.T).astype(bf)
        m['gidx'] = np.ascontiguousarray(
            gi.reshape(NT, 128).T).astype(np.int32)
        g16 = np.empty((128, NCH * (CR // 16)), np.int16)
        for cc in range(NCH):
            blk = gi[cc * CR:(cc + 1) * CR].reshape(CR // 16, 16).T  # (16, s)
            g16[:, cc * (CR // 16):(cc + 1) * (CR // 16)] = np.tile(
                blk, (8, 1))
        m['gidx16'] = g16
        emt = em.reshape(NT, 128).T
        m['emo'] = np.ascontiguousarray(
            (emt - 1.0) * MASK_OFF * 1.4426950408889634).astype(f32)
        m['emb'] = np.ascontiguousarray(emt).astype(bf)
        nsh = node[rs, :]
        m['node_sh_t'] = ktile(nsh.T).astype(bf)
        m['node_sh_f'] = ktile(nsh.T).astype(f32)
        in_maps.append(m)
    return in_maps


LAST_EXEC_NS = None
LAST_RESULTS = None


def _install_ntff_shim():
    """Provide antenv.axon_hooks (missing in this image) so trace=True works."""
    import sys
    import types
    import contextlib
    import ctypes
    try:
        from antenv.axon_hooks import get_axon_ntff_profile_hook  # noqa
        return
    except ImportError:
        pass
    so_path = "/opt/axon/libaxon_pjrt.so"
    try:
        lib = ctypes.CDLL(so_path)
    except OSError:
        lib = None
    hook = None
    if lib is not None and hasattr(lib, "axon_start_nrt_profile"):
        lib.axon_start_nrt_profile.argtypes = [
            ctypes.POINTER(ctypes.c_int64), ctypes.c_size_t]
        lib.axon_start_nrt_profile.restype = ctypes.c_int64
        lib.axon_stop_nrt_profile.argtypes = [ctypes.c_char_p]
        lib.axon_stop_nrt_profile.restype = ctypes.c_int64

        @contextlib.contextmanager
        def _hook(output_dir, device_ids):
            import jax
            jax.devices()
            if device_ids:
                ids = (ctypes.c_int64 * len(device_ids))(*device_ids)
                rc = lib.axon_start_nrt_profile(ids, len(device_ids))
            else:
                rc = lib.axon_start_nrt_profile(None, 0)
            if rc != 0:
                raise RuntimeError(f"axon_start_nrt_profile rc={rc}")
            try:
                yield
            finally:
                n = lib.axon_stop_nrt_profile(str(output_dir).encode())
                print(f"profile: {n} file(s) written to {output_dir}")

        hook = _hook
    mod = types.ModuleType("antenv.axon_hooks")
    mod.get_axon_ntff_profile_hook = lambda: hook
    mod.set_axon_ntff_profile_hook = lambda h: None
    sys.modules["antenv.axon_hooks"] = mod


def _run_device(in_maps, trace=False, tmpdir=None):
    global LAST_EXEC_NS, LAST_RESULTS
    if trace:
        try:
            _install_ntff_shim()
        except Exception:
            trace = False
    from concourse.bass_utils import run_bass_kernel_spmd
    if 'nc' not in _CACHE:
        _CACHE['nc'] = _build_nc()
    try:
        res = run_bass_kernel_spmd(
            _CACHE['nc'], in_maps, core_ids=list(range(NC)), trace=trace,
            tmpdir=tmpdir)
    except Exception:
        if not trace:
            raise
        res = run_bass_kernel_spmd(
            _CACHE['nc'], in_maps, core_ids=list(range(NC)), trace=False)
    if res.exec_time_ns:
        LAST_EXEC_NS = res.exec_time_ns
    LAST_RESULTS = res
    return res.results


def kernel(**inputs) -> np.ndarray:
    import os
    prep = _prep(inputs)
    if prep is None:
        raise RuntimeError("mask_bw != 1 unsupported")
    trace = os.environ.get("ATOM_TRACE", "0") == "1"
    outs = _run_device(prep, trace=trace)
    full = np.empty((B * N, DIM), np.float32)
    for c in range(NC):
        full[c * R:(c + 1) * R, :] = outs[c]['out_t'].T
    return full.reshape(B, N, DIM)



# revision 3
# speedup vs baseline: 1.4690x; 1.4690x over previous
"""AtomDecoderLayer (GNN message passing) on 8 trn2 NeuronCores via Bass/Tile.

Sharding: flattened (batch, node) rows 2*1024 = 2048 -> 8 shards of 256 nodes
(8192 edge-rows each). Weights replicated. The edge_index gather runs on
device: every core computes the full node_msg table (2048, 256) -> DRAM
scratch, then indirect-DMA-gathers its 8192 rows (one batched xbar transpose
per chunk moves them into feature-major layout).

Layout: activations are feature-on-partition ("T space"); every dense matmul
is lhsT=weight (K,M<=128 slices), rhs=activation^T, accumulating K tiles in
PSUM. fp32 accumulation. The edge-heavy matmuls (edge proj, msg MLP W1/W2)
run in fp8e4m3 with DoubleRow perf mode (2 k-tiles per matmul); their
weights are pre-scaled x64 on the host so 0.02-magnitude weights stay out of
fp8 subnormals, and the matching node src/tgt projections are scaled x64 in
bf16 (exact) so the shared msg0 PSUM is uniformly x64. LayerNorm is scale
invariant so the x64 washes out in xln; gelu/identity activations descale by
1/64 via the ACT scale input. LN gain/bias fold into the following weight on
the host; LN stats come from (1/256)-lhsT matmuls; rstd runs on DVE in bf16
via the int16 bit trick + 1 Newton step. The target-node projection is
injected into the msg0 PSUM by an extra matmul against a constant
node->edge indicator. The GAT attention einsum uses a block-diagonal rhs
trick: per 128-edge-row tile (4 nodes x 32 neighbors), lhsT = msg rows (row
space), rhs = A where A[:, 17g:17g+16] = exp(ab) of node g and
A[:, 17g+16] = edge mask; the matmul yields both the unnormalized
attention-weighted message sums (m) and the masked mean-pool sums, and a
ones-lhsT matmul over A yields softmax + pool denominators. W_attn_bias is
pre-scaled by log2(e) so exp becomes a bf16 2^y evaluated on DVE (round via
+192 magic, quadratic 2^f poly, exponent bits built in int16). og is a small
per-head projection of m (replaces the big msg @ W_gat_value matmul). The
attention tail of chunk c is pipelined into iteration c+1. zt uses the
Prelu activation (parametric_relu lives in every ACT table set, unlike
leaky_relu) so the chunk loop needs no ACT table switches at all.

Host does layout transforms / casts / index arithmetic only; all FLOPs run
on device. Self-contained: hardcodes b=2, n=1024, k=32, dim=512, pdim=256,
msg=256, H=16, D=64.
"""

import numpy as np

B, N, K = 2, 1024, 32
DIM, PD, MD = 512, 256, 256
H, DH = 16, 64
NC = 8
R = B * N // NC            # 256 nodes per core
RK = R * K                 # 8192 edge rows per core
NT = RK // 128             # 64 row-tiles per core
NCH = 8                    # chunks per core
CT = NT // NCH             # 8 row-tiles per chunk
CR = CT * 128              # 1024 edge rows per chunk
CNODE = CR // K            # 32 nodes per chunk
NS = CR // 512             # 2 512-col subtiles per chunk
LN_EPS = 1e-5
NEG = 0.01
MASK_OFF = 60.0
LOG2E = 1.4426950408889634
WS = 64.0                  # fp8 weight pre-scale (power of two)

_CACHE = {}


# =========================================================================
# device kernel (Bass IR)
# =========================================================================
def _build_nc(debug=False):
    import concourse.bacc as bacc
    import concourse.bass as bass
    import concourse.mybir as mybir
    import concourse.tile as tile

    dt = mybir.dt
    AF = mybir.ActivationFunctionType
    OP = mybir.AluOpType
    PM = mybir.MatmulPerfMode
    f32, bf16, f8 = dt.float32, dt.bfloat16, dt.float8e4

    nc = bacc.Bacc(None, target_bir_lowering=False)

    def din(name, shape, d=bf16):
        return nc.dram_tensor(name, shape, d, kind="ExternalInput")

    # per-core inputs
    edge_t = din("edge_t", [PD, RK], f8)             # edge_repr^T (fp8)
    gidx = din("gidx", [128, NT], dt.int32)          # gather row ids [p, t]
    emo = din("emo", [128, NT])                      # (em-1)*60*log2e bf16
    emb = din("emb", [128, NT])                      # em (bf16) [p, t]
    node_sh_t = din("node_sh_t", [128, 4 * R])       # node shard^T bf16
    node_sh_f = din("node_sh_f", [128, 4 * R], f32)
    # replicated inputs
    node_t = din("node_t", [128, 4 * B * N])         # full node_repr^T bf16
    w_src_r = din("w_src_r", [128, 4 * MD])          # rhs layout, x64
    w_edge_l = din("w_edge_l", [128, 2 * MD], f8)    # x64
    w_tgt_r = din("w_tgt_r", [128, 4 * MD])          # rhs layout, x64
    w1_l = din("w1_l", [128, 2 * 4 * MD], f8)        # x64, LN-g folded
    w2_l = din("w2_l", [128, 8 * MD], f8)            # x64
    wmsg_l = din("wmsg_l", [128, 2 * MD])
    wab_l = din("wab_l", [128, 2 * H])               # x log2e
    wv_l = din("wv_l", [128, 2 * H * DH])
    wgate_l = din("wgate_l", [128, 4 * MD])
    wggate_l = din("wggate_l", [128, 4 * H * DH])
    wout_l = din("wout_l", [128, 2 * DIM])
    wgo_l = din("wgo_l", [128, 8 * DIM])
    w1n_l = din("w1n_l", [128, 4 * 4 * DIM])
    w2n_l = din("w2n_l", [128, 16 * DIM])
    sel_l = din("sel_l", [H, 8 * 128])               # head-broadcast indicator
    ebk_l = din("ebk_l", [CNODE, CR])                # node->edge indicator
    biases = din("biases", [128, 40], f32)           # packed bias columns

    out_t = nc.dram_tensor("out_t", [DIM, R], f32, kind="ExternalOutput")

    dbg = {}
    if debug:
        for nm, shape, d in [
            ("d_gat", [128, 2 * CR], bf16), ("d_m0b", [128, 2 * CR], bf16),
            ("d_xln", [128, 2 * CR], f8), ("d_msg", [128, 2 * CR], bf16),
            ("d_abr", [128, CT * H], bf16), ("d_abm", [128, CT * H], bf16),
            ("d_A", [128, CT * 68], bf16), ("d_mrow", [128, 2 * CR], bf16),
            ("d_mT", [128, 2 * NT * 68], bf16), ("d_den", [1, NT * 68], f32),
            ("d_s16", [H, R], f32), ("d_g1", [128, 2 * R], f32),
            ("d_o", [128, 2 * R], bf16), ("d_og", [128, 8 * R], bf16),
            ("d_x1f", [128, 4 * R], f32), ("d_tgt", [CNODE, NCH * 2 * 128], bf16),
            ("d_rstd", [128, CR], bf16), ("d_h1", [128, 8 * 512], f8),
        ]:
            dbg[nm] = nc.dram_tensor(nm, shape, d, kind="ExternalOutput")

    # packed bias column offsets
    B1T, B2M, BGA, BGG, B1NT, B2N = 0, 8, 10, 12, 20, 36

    with tile.TileContext(nc) as tc:
        with (
            tc.tile_pool(name="per", bufs=1) as per,
            tc.tile_pool(name="wts", bufs=1) as wts,
            tc.tile_pool(name="dram", bufs=1, space="DRAM") as dpool,
        ):
            def load(t, shape, d=bf16, pool=wts):
                s = pool.tile(shape, d, tag=t.name)
                nc.sync.dma_start(s[:], t[:])
                return s

            node_sh_s = load(node_sh_t, [128, 4 * R], pool=per)
            node_shf_s = load(node_sh_f, [128, 4 * R], f32, pool=per)
            gidx_s = load(gidx, [128, NT], dt.int32, pool=per)
            emo_s = load(emo, [128, NT], pool=per)
            emb_s = load(emb, [128, NT], pool=per)
            bias_s = load(biases, [128, 40], f32, pool=per)

            def bia(base, m):
                return bias_s[:, base + m:base + m + 1]

            def rsqrt_dve(pool, ap, tag):
                """In-place y = 1/sqrt(ap) on DVE fp32 (bit trick + 1 Newton)."""
                shp = list(ap.shape)
                y0 = pool.tile(shp, f32, tag=tag + "_y0")
                t1 = pool.tile(shp, f32, tag=tag + "_t1")
                vi = ap.bitcast(dt.int32)
                y0i = y0[:].bitcast(dt.int32)
                nc.vector.tensor_scalar(
                    out=y0i, in0=vi, scalar1=1, scalar2=None,
                    op0=OP.arith_shift_right)
                nc.vector.tensor_scalar(
                    out=y0i, in0=y0i, scalar1=-1, scalar2=0x5f3759df,
                    op0=OP.mult, op1=OP.add)
                nc.vector.tensor_tensor(out=t1[:], in0=ap, in1=y0[:],
                                        op=OP.mult)
                nc.vector.tensor_tensor(out=t1[:], in0=t1[:], in1=y0[:],
                                        op=OP.mult)
                nc.vector.tensor_scalar(
                    out=t1[:], in0=t1[:], scalar1=-0.5, scalar2=1.5,
                    op0=OP.mult, op1=OP.add)
                nc.vector.tensor_tensor(out=ap, in0=y0[:], in1=t1[:],
                                        op=OP.mult)

            def rsqrt16(pool, ap, tag):
                """In-place y = 1/sqrt(ap) on DVE bf16 (int16 trick + Newton)."""
                shp = list(ap.shape)
                y0 = pool.tile(shp, bf16, tag=tag + "_y0")
                t1 = pool.tile(shp, bf16, tag=tag + "_t1")
                # int16 shifts are invalid ISA on DVE; the ALU computes in
                # fp32, so bits*-0.5 + magic == magic - (bits>>1) up to RNE.
                vi = ap.bitcast(dt.int16)
                y0i = y0[:].bitcast(dt.int16)
                nc.vector.tensor_scalar(
                    out=y0i, in0=vi, scalar1=-0.5, scalar2=0x5f37,
                    op0=OP.mult, op1=OP.add)
                nc.vector.tensor_tensor(out=t1[:], in0=ap, in1=y0[:],
                                        op=OP.mult)
                nc.vector.tensor_tensor(out=t1[:], in0=t1[:], in1=y0[:],
                                        op=OP.mult)
                nc.vector.tensor_scalar(
                    out=t1[:], in0=t1[:], scalar1=-0.5, scalar2=1.5,
                    op0=OP.mult, op1=OP.add)
                nc.vector.tensor_tensor(out=ap, in0=y0[:], in1=t1[:],
                                        op=OP.mult)

            ones_s = per.tile([128, 128], bf16, tag="ones")
            nc.vector.memset(ones_s[:], 1.0)
            ones_md = per.tile([128, 128], bf16, tag="ones_md")
            nc.vector.memset(ones_md[:], 1.0 / MD)

            g1 = per.tile([128, 2 * R], f32, tag="g1")
            g2 = per.tile([128, 8 * R], f32, tag="g2")

            table = dpool.tile([B * N, MD], bf16, tag="table")

            # persistent activations
            tgt32 = per.tile([CNODE, NCH * 2 * 128], bf16, tag="tgt32")
            mT_s = per.tile([128, 2 * NT * 68], bf16, tag="mT")   # einsum out
            den_s = per.tile([1, NT * 68], f32, tag="den")        # denominators
            den_v = den_s[:1, :].rearrange("o (c T g) -> o c T g", c=17, g=4)

            # ---------------- phase 0: node_msg table + tgt --------------
            with (
                tc.tile_pool(name="p0s", bufs=2) as p0s,
                tc.tile_pool(name="p0p", bufs=4, space="PSUM") as p0p,
            ):
                node_t_s = load(node_t, [128, 4 * B * N], pool=p0s)
                w_src_s = load(w_src_r, [128, 4 * MD], pool=p0s)
                w_tgt_s = load(w_tgt_r, [128, 4 * MD], pool=p0s)
                wgate_s = load(wgate_l, [128, 4 * MD], pool=p0s)
                wggate_s = load(wggate_l, [128, 4 * H * DH], pool=p0s)
                # 4 batched table writes (4 node-tiles each)
                for g in range(4):
                    sb = p0s.tile([128, 4 * MD], bf16, tag="tbev")
                    for i in range(4):
                        s = g * 4 + i
                        p = p0p.tile([128, MD], f32, tag="pt")
                        c0 = s * 128
                        for k in range(4):
                            nc.tensor.matmul(
                                p[:],
                                node_t_s[:, k * B * N + c0:k * B * N + c0 + 128],
                                w_src_s[:, k * MD:(k + 1) * MD],
                                start=(k == 0), stop=(k == 3))
                        nc.vector.tensor_copy(sb[:, i * MD:(i + 1) * MD], p[:])
                    nc.sync.dma_start(
                        table[g * 512:(g + 1) * 512, :].rearrange(
                            "(k p) m -> p k m", k=4),
                        sb[:, :].rearrange("p (k m) -> p k m", k=4))

                # tgt in row space, 32 nodes per chunk at partitions 0-31
                for c in range(NCH):
                    p = p0p.tile([CNODE, MD], f32, tag="pg")
                    for k in range(4):
                        nc.tensor.matmul(
                            p[:],
                            node_sh_s[:, k * R + c * CNODE:
                                      k * R + c * CNODE + CNODE],
                            w_tgt_s[:, k * MD:(k + 1) * MD],
                            start=(k == 0), stop=(k == 3))
                    nc.vector.tensor_copy(
                        tgt32[:, :].rearrange("p (c j e) -> p c (j e)", j=2,
                                              c=NCH)[:, c, :], p[:])
                if debug:
                    nc.sync.dma_start(dbg["d_tgt"][:], tgt32[:])

                # gates (raw; pool-denominator folded in at the tail)
                for m in range(2):
                    p = p0p.tile([128, R], f32, tag="pt")
                    for k in range(4):
                        nc.tensor.matmul(
                            p[:],
                            wgate_s[:, k * MD + m * 128:k * MD + m * 128 + 128],
                            node_sh_s[:, k * R:(k + 1) * R],
                            start=(k == 0), stop=(k == 3))
                    nc.scalar.activation(g1[:, m * R:(m + 1) * R], p[:],
                                         AF.Sigmoid, bias=bia(BGA, m))
                for m in range(8):
                    p = p0p.tile([128, R], f32, tag="pt")
                    for k in range(4):
                        nc.tensor.matmul(
                            p[:],
                            wggate_s[:, k * H * DH + m * 128:
                                     k * H * DH + m * 128 + 128],
                            node_sh_s[:, k * R:(k + 1) * R],
                            start=(k == 0), stop=(k == 3))
                    nc.scalar.activation(g2[:, m * R:(m + 1) * R], p[:],
                                         AF.Sigmoid, bias=bia(BGG, m))

            w_edge_s = load(w_edge_l, [128, 2 * MD], f8)
            w1_s = load(w1_l, [128, 2 * 4 * MD], f8)
            w2_s = load(w2_l, [128, 8 * MD], f8)
            wmsg_s = load(wmsg_l, [128, 2 * MD])
            wab_s = load(wab_l, [128, 2 * H])
            ebk_s = load(ebk_l, [CNODE, CR])
            wv_s = load(wv_l, [128, 2 * H * DH])
            sel_s = load(sel_l, [H, 8 * 128])
            wout_s = load(wout_l, [128, 2 * DIM])
            wgo_s = load(wgo_l, [128, 8 * DIM])

            we3 = w_edge_s[:, :].rearrange("p (k m) -> p k m", k=2)
            w13 = w1_s[:, :].rearrange("p (k m) -> p k m", k=2)
            w23 = w2_s[:, :].rearrange("p (k m) -> p k m", k=8)

            # ---------------- chunk loop (attention tail pipelined) ------
            with (
                tc.tile_pool(name="ch1", bufs=1) as ch1,
                tc.tile_pool(name="ch2", bufs=2) as ch2,
                tc.tile_pool(name="ch3", bufs=3) as ch3,
                tc.tile_pool(name="cpw", bufs=3, space="PSUM") as cpw,
                tc.tile_pool(name="cpm", bufs=2, space="PSUM") as cpm,
                tc.tile_pool(name="cpe", bufs=3, space="PSUM") as cpe,
            ):
                mT4 = mT_s[:, :].rearrange("p (j t c) -> p j t c", j=2, t=NT)

                def front(c):
                    c0 = c * CR
                    ech = ch2.tile([128, 2 * CR], f8, tag="ech")
                    for k in range(2):
                        nc.sync.dma_start(
                            ech[:, k * CR:(k + 1) * CR],
                            edge_t[k * 128:(k + 1) * 128, c0:c0 + CR])
                    ech3 = ech[:, :].rearrange("p (k x) -> p k x", k=2)

                    # gather (row space, one indirect DMA), xbar to T space
                    grow = ch3.tile([128, CT * MD], bf16, tag="grow")
                    for t in range(CT):
                        nc.gpsimd.indirect_dma_start(
                            out=grow[:, t * MD:(t + 1) * MD], out_offset=None,
                            in_=table[:],
                            in_offset=bass.IndirectOffsetOnAxis(
                                ap=gidx_s[:, c * CT + t:c * CT + t + 1],
                                axis=0))
                    gat = ch2.tile([128, 2 * CR], bf16, tag="gat")
                    nc.sync.dma_start(
                        gat[:, :].rearrange("p (t j e) -> p (t j) e", j=2,
                                            e=128),
                        grow[:], transpose=True)
                    gat4 = gat[:, :].rearrange("p (t j e) -> p t j e",
                                               j=2, t=CT)
                    if debug and c == 0:
                        nc.sync.dma_start(dbg["d_gat"][:], gat[:])

                    # msg0 = edge@We (fp8 DR) + tgt-indicator-mm + gath (DVE)
                    m0b = ch2.tile([128, 2 * CR], bf16, tag="m0b")
                    for m in range(2):
                        for s in range(NS):
                            p = cpm.tile([128, 512], f32, tag="pm")
                            nc.tensor.matmul(
                                p[:],
                                we3[:, 0:2, m * 128:m * 128 + 128],
                                ech3[:, 0:2, s * 512:s * 512 + 512],
                                start=True, stop=False,
                                perf_mode=PM.DoubleRow)
                            nc.tensor.matmul(
                                p[:],
                                tgt32[:, (c * 2 + m) * 128:
                                      (c * 2 + m) * 128 + 128],
                                ebk_s[:, s * 512:s * 512 + 512],
                                start=False, stop=True)
                            p3t = p[:].rearrange("p (t e) -> p t e", e=128)
                            m3 = m0b[:, m * CR + s * 512:m * CR + s * 512
                                     + 512].rearrange("p (t e) -> p t e",
                                                      e=128)
                            nc.vector.tensor_tensor(
                                out=m3, in0=p3t,
                                in1=gat4[:, 4 * s:4 * s + 4, m, :],
                                op=OP.add)
                    if debug and c == 0:
                        nc.sync.dma_start(dbg["d_m0b"][:], m0b[:])

                    # LN stats (x64 units; LN is scale invariant, eps dropped)
                    sq = ch1.tile([128, 2 * CR], bf16, tag="sq")
                    for m in range(2):
                        nc.vector.tensor_tensor(
                            out=sq[:, m * CR:(m + 1) * CR],
                            in0=m0b[:, m * CR:(m + 1) * CR],
                            in1=m0b[:, m * CR:(m + 1) * CR], op=OP.mult)
                    mu = ch2.tile([128, CR], bf16, tag="mu")
                    st = ch2.tile([128, CR], bf16, tag="st")
                    for src, dst in ((m0b, mu), (sq, st)):
                        for s in range(NS):
                            p = cpm.tile([128, 512], f32, tag="pm")
                            for k in range(2):
                                nc.tensor.matmul(
                                    p[:], ones_md[:],
                                    src[:, k * CR + s * 512:
                                        k * CR + s * 512 + 512],
                                    start=(k == 0), stop=(k == 1))
                            nc.any.tensor_copy(
                                dst[:, s * 512:(s + 1) * 512], p[:])
                    mu2 = ch1.tile([128, CR], bf16, tag="mu2")
                    nc.vector.tensor_tensor(out=mu2[:], in0=mu[:], in1=mu[:],
                                            op=OP.mult)
                    nc.vector.tensor_tensor(out=st[:], in0=st[:], in1=mu2[:],
                                            op=OP.subtract)
                    rsqrt16(ch1, st[:], "rs")
                    if debug and c == 0:
                        nc.sync.dma_start(dbg["d_rstd"][:], st[:])
                    xln = ch2.tile([128, 2 * CR], f8, tag="xln")
                    xt = ch1.tile([128, CR], bf16, tag="xt")
                    for m in range(2):
                        sl = slice(m * CR, (m + 1) * CR)
                        nc.vector.tensor_tensor(out=xt[:], in0=m0b[:, sl],
                                                in1=mu[:], op=OP.subtract)
                        nc.vector.tensor_tensor(out=xln[:, sl], in0=xt[:],
                                                in1=st[:], op=OP.mult)
                    xln3 = xln[:, :].rearrange("p (k x) -> p k x", k=2)
                    if debug and c == 0:
                        nc.sync.dma_start(dbg["d_xln"][:], xln[:])

                    # msg MLP (fp8 DoubleRow, weights x64, descale in ACT)
                    msg = ch2.tile([128, 2 * CR], bf16, tag="msg")
                    for s in range(NS):
                        h1 = ch2.tile([128, 8 * 512], f8, tag="h1")
                        for m in range(8):
                            p = cpw.tile([128, 512], f32, tag="pw")
                            nc.tensor.matmul(
                                p[:],
                                w13[:, 0:2, m * 128:m * 128 + 128],
                                xln3[:, 0:2, s * 512:s * 512 + 512],
                                start=True, stop=True,
                                perf_mode=PM.DoubleRow)
                            nc.scalar.activation(
                                h1[:, m * 512:(m + 1) * 512], p[:], AF.Gelu,
                                bias=bia(B1T, m), scale=1.0 / WS)
                        if debug and c == 0 and s == 0:
                            nc.sync.dma_start(dbg["d_h1"][:], h1[:])
                        h13 = h1[:, :].rearrange("p (k x) -> p k x", k=8)
                        for m2 in range(2):
                            p = cpw.tile([128, 512], f32, tag="pw")
                            for t in range(4):
                                nc.tensor.matmul(
                                    p[:],
                                    w23[:, 2 * t:2 * t + 2,
                                        m2 * 128:m2 * 128 + 128],
                                    h13[:, 2 * t:2 * t + 2, :],
                                    start=(t == 0), stop=(t == 3),
                                    perf_mode=PM.DoubleRow)
                            nc.scalar.activation(
                                msg[:, m2 * CR + s * 512:
                                    m2 * CR + s * 512 + 512],
                                p[:], AF.Identity, bias=bia(B2M, m2),
                                scale=1.0 / WS)
                    if debug and c == 0:
                        nc.sync.dma_start(dbg["d_msg"][:], msg[:])

                    # z = prelu(msg @ Wmsg); ab = z @ (Wab * log2e)
                    zt = ch1.tile([128, 2 * CR], bf16, tag="zt")
                    for m in range(2):
                        for s in range(NS):
                            p = cpw.tile([128, 512], f32, tag="pw")
                            for k in range(2):
                                nc.tensor.matmul(
                                    p[:],
                                    wmsg_s[:, k * MD + m * 128:
                                           k * MD + m * 128 + 128],
                                    msg[:, k * CR + s * 512:
                                        k * CR + s * 512 + 512],
                                    start=(k == 0), stop=(k == 1))
                            nc.scalar.activation(
                                zt[:, m * CR + s * 512:m * CR + s * 512 + 512],
                                p[:], AF.Prelu, alpha=NEG)
                    abt = ch1.tile([H, CR], bf16, tag="abt")
                    for s in range(NS):
                        pfull = cpe.tile([128, 512], f32, tag="pback")
                        p = pfull[:H, :]
                        for k in range(2):
                            nc.tensor.matmul(
                                p[:], wab_s[:, k * H:(k + 1) * H],
                                zt[:, k * CR + s * 512:k * CR + s * 512 + 512],
                                start=(k == 0), stop=(k == 1))
                        nc.vector.tensor_copy(abt[:, s * 512:(s + 1) * 512],
                                              p[:])
                    abr = ch2.tile([128, CT * H], bf16, tag="abr")
                    nc.sync.dma_start(
                        abr[:, :].rearrange("p (t h) -> p t h", h=H),
                        abt[:], transpose=True)
                    if debug and c == 0:
                        nc.sync.dma_start(dbg["d_abr"][:], abr[:])

                    # msg -> row space (one xbar transpose)
                    mrow = ch2.tile([128, 2 * CR], bf16, tag="mrow")
                    nc.sync.dma_start(
                        mrow[:, :].rearrange("p (q e) -> p q e", e=128),
                        msg[:], transpose=True)
                    if debug and c == 0:
                        nc.sync.dma_start(dbg["d_mrow"][:], mrow[:])
                    mrow4 = mrow[:, :].rearrange("p (j t e) -> p j t e",
                                                 j=2, t=CT)
                    # abm = 2^(ab + emo) on DVE, all bf16:
                    # round via +192 magic; quad 2^f poly; exponent via int16
                    W = CT * H
                    MAGIC = 192.0
                    ey = ch1.tile([128, W], bf16, tag="ey")
                    ez = ch1.tile([128, W], bf16, tag="ez")
                    en = ch1.tile([128, W], bf16, tag="en")
                    ef = ch1.tile([128, W], bf16, tag="ef")
                    ep = ch1.tile([128, W], bf16, tag="ep")
                    e3 = emo_s[:, c * CT:(c + 1) * CT].to_broadcast(
                        [128, CT, H])
                    nc.vector.tensor_tensor(
                        out=ey[:].rearrange("p (t h) -> p t h", h=H),
                        in0=abr[:, :].rearrange("p (t h) -> p t h", h=H),
                        in1=e3, op=OP.add)
                    nc.vector.tensor_scalar_add(ez[:], ey[:], MAGIC)
                    nc.vector.tensor_scalar_sub(en[:], ez[:], MAGIC)
                    nc.vector.tensor_tensor(out=ef[:], in0=ey[:], in1=en[:],
                                            op=OP.subtract)
                    # p = 1 + f(ln2 + f c2)
                    nc.vector.tensor_scalar(
                        out=ep[:], in0=ef[:], scalar1=0.2416,
                        scalar2=0.6931472, op0=OP.mult, op1=OP.add)
                    nc.vector.tensor_tensor(out=ep[:], in0=ep[:], in1=ef[:],
                                            op=OP.mult)
                    nc.vector.tensor_scalar_add(ep[:], ep[:], 1.0)
                    # 2^n bits: (n+127)*128 as bf16 value -> int16 -> bitcast
                    nc.vector.tensor_scalar(
                        out=ez[:], in0=en[:], scalar1=128.0,
                        scalar2=16256.0, op0=OP.mult, op1=OP.add)
                    ei = ch1.tile([128, W], dt.int16, tag="ei")
                    nc.vector.tensor_copy(ei[:], ez[:])
                    abm = ch1.tile([128, CT * H], bf16, tag="abm")
                    abm3 = abm[:, :].rearrange("p (t h) -> p t h", h=H)
                    nc.vector.tensor_tensor(out=abm[:], in0=ep[:],
                                            in1=ei[:].bitcast(bf16),
                                            op=OP.mult)
                    if debug and c == 0:
                        nc.sync.dma_start(dbg["d_abm"][:], abm[:])

                    A = ch1.tile([128, CT * 68], bf16, tag="A")
                    nc.vector.memset(A[:], 0.0)
                    A3 = A[:, :].rearrange("p (t c) -> p t c", c=68)
                    em3 = emb_s[:, c * CT:(c + 1) * CT].to_broadcast(
                        [128, CT, 1])
                    for g in range(4):
                        rs = slice(32 * g, 32 * g + 32)
                        nc.vector.tensor_copy(A3[rs, :, 17 * g:17 * g + 16],
                                              abm3[rs, :, :])
                        nc.vector.tensor_copy(
                            A3[rs, :, 17 * g + 16:17 * g + 17], em3[rs, :, :])
                    if debug and c == 0:
                        nc.sync.dma_start(dbg["d_A"][:], A[:])

                    # einsum: batch 7 tiles per PSUM bank, 1 copy each m
                    for m in range(2):
                        pm_ = cpe.tile([128, 512], f32, tag="pback")
                        for t in range(7):
                            nc.tensor.matmul(
                                pm_[:, t * 68:(t + 1) * 68],
                                mrow4[:, m, t, :], A3[:, t, :],
                                start=True, stop=True)
                        nc.any.tensor_copy(
                            mT4[:, m, c * CT:c * CT + 7, :],
                            pm_[:, :476].rearrange("p (t c) -> p t c", c=68))
                    p7 = cpe.tile([128, 512], f32, tag="pback")
                    for m in range(2):
                        nc.tensor.matmul(
                            p7[:, m * 68:m * 68 + 68],
                            mrow4[:, m, 7, :], A3[:, 7, :],
                            start=True, stop=True)
                    for m in range(2):
                        nc.any.tensor_copy(mT4[:, m, c * CT + 7, :],
                                           p7[:, m * 68:m * 68 + 68])
                    for t0, nt_ in ((0, 7), (7, 1)):
                        w = nt_ * 68
                        p = cpe.tile([128, 512], f32, tag="pback")
                        nc.tensor.matmul(p[:, :w], ones_s[:],
                                         A[:, t0 * 68:t0 * 68 + w],
                                         start=True, stop=True)
                        nc.scalar.copy(
                            den_v[:, :, c * CT + t0:c * CT + t0 + nt_, :]
                            .rearrange("o c T g -> o T g c"),
                            p[:1, :w].rearrange("o (t g c) -> o t g c",
                                                g=4, c=17))

                for c in range(NCH):
                    front(c)

            # ---------------- tail ---------------------------------------
            with (
                tc.tile_pool(name="tl", bufs=1) as tl,
                tc.tile_pool(name="tp", bufs=4, space="PSUM") as tp,
                tc.tile_pool(name="tp2", bufs=2, space="PSUM") as tp2,
            ):
                w1n_s = load(w1n_l, [128, 4 * 4 * DIM], pool=tl)
                w2n_s = load(w2n_l, [128, 16 * DIM], pool=tl)
                if debug:
                    nc.sync.dma_start(dbg["d_mT"][:], mT_s[:])
                    nc.sync.dma_start(dbg["d_den"][:], den_s[:])
                # s16(h, node) = 1/softmax_den; sE(node) = 1/(pool_den+1e-6)
                s16 = tl.tile([H, R], f32, tag="s16")
                nc.sync.dma_start(s16[:], den_s[:1, 0:H * R])
                rsqrt_dve(tl, s16[:], "rs16")
                nc.vector.tensor_tensor(out=s16[:], in0=s16[:], in1=s16[:],
                                        op=OP.mult)
                if debug:
                    nc.sync.dma_start(dbg["d_s16"][:], s16[:])
                s16b = tl.tile([H, R], bf16, tag="s16b")
                nc.vector.tensor_copy(s16b[:], s16[:])
                sE = tl.tile([1, R], f32, tag="sE")
                nc.sync.dma_start(sE[:], den_s[:1, H * R:17 * R])
                nc.vector.tensor_scalar_add(sE[:], sE[:], 1e-6)
                rsqrt_dve(tl, sE[:], "rsE")
                nc.vector.tensor_tensor(out=sE[:], in0=sE[:], in1=sE[:],
                                        op=OP.mult)
                sEb = tl.tile([1, R], bf16, tag="sEb")
                nc.vector.tensor_copy(sEb[:], sE[:])
                sE_ps = tp2.tile([128, R], f32, tag="psE")
                nc.tensor.matmul(sE_ps[:], ones_s[:1, :], sEb[:],
                                 start=True, stop=True)

                # fold pool denominator into g1
                for m in range(2):
                    nc.vector.tensor_tensor(out=g1[:, m * R:(m + 1) * R],
                                            in0=g1[:, m * R:(m + 1) * R],
                                            in1=sE_ps[:], op=OP.mult)
                if debug:
                    nc.sync.dma_start(dbg["d_g1"][:], g1[:])

                mT5 = mT_s[:, :].rearrange("p (j t g c) -> p j t g c",
                                           j=2, t=NT, g=4)
                # o^T gated
                o_s = tl.tile([128, 2 * R], bf16, tag="o")
                for m in range(2):
                    nc.vector.tensor_tensor(
                        out=o_s[:, m * R:(m + 1) * R].rearrange(
                            "p (t g) -> p t g", g=4),
                        in0=mT5[:, m, :, :, 16],
                        in1=g1[:, m * R:(m + 1) * R].rearrange(
                            "p (t g) -> p t g", g=4),
                        op=OP.mult)
                if debug:
                    nc.sync.dma_start(dbg["d_o"][:], o_s[:])

                # og: per-head projection of m, then softmax scale and gate
                og_s = tl.tile([128, 8 * R], bf16, tag="og")
                for tpi in range(8):
                    p = tp.tile([128, R], f32, tag="pt")
                    for hh in range(2):
                        h = 2 * tpi + hh
                        for k in range(2):
                            nc.tensor.matmul(
                                p[64 * hh:64 * hh + 64, :],
                                wv_s[:, k * H * DH + h * DH:
                                     k * H * DH + h * DH + DH],
                                mT5[:, k, :, :, h].rearrange(
                                    "p t g -> p (t g)"),
                                start=(k == 0), stop=(k == 1))
                    sp = tp2.tile([128, R], f32, tag="ps16")
                    nc.tensor.matmul(sp[:], sel_s[:, tpi * 128:(tpi + 1) * 128],
                                     s16b[:], start=True, stop=True)
                    sg = tl.tile([128, R], f32, tag="sg")
                    nc.vector.tensor_tensor(out=sg[:], in0=sp[:],
                                            in1=g2[:, tpi * R:(tpi + 1) * R],
                                            op=OP.mult)
                    nc.vector.tensor_tensor(
                        out=og_s[:, tpi * R:(tpi + 1) * R], in0=p[:],
                        in1=sg[:], op=OP.mult)
                if debug:
                    nc.sync.dma_start(dbg["d_og"][:], og_s[:])

                # dh = W_out.T @ o + W_go.T @ og;  x1 = node + dh
                x1f = tl.tile([128, 4 * R], f32, tag="x1f")
                x1b = tl.tile([128, 4 * R], bf16, tag="x1b")
                for m in range(4):
                    p = tp.tile([128, R], f32, tag="pt")
                    for k in range(2):
                        nc.tensor.matmul(
                            p[:],
                            wout_s[:, k * DIM + m * 128:k * DIM + m * 128 + 128],
                            o_s[:, k * R:(k + 1) * R],
                            start=(k == 0), stop=False)
                    for k in range(8):
                        nc.tensor.matmul(
                            p[:],
                            wgo_s[:, k * DIM + m * 128:k * DIM + m * 128 + 128],
                            og_s[:, k * R:(k + 1) * R],
                            start=False, stop=(k == 7))
                    sl = slice(m * R, (m + 1) * R)
                    nc.vector.tensor_tensor(out=x1f[:, sl], in0=p[:],
                                            in1=node_shf_s[:, sl], op=OP.add)
                    nc.vector.tensor_copy(x1b[:, sl], x1f[:, sl])
                if debug:
                    nc.sync.dma_start(dbg["d_x1f"][:], x1f[:])

                # node MLP with LN folded into W1n (bf16)
                sqn = tl.tile([128, 4 * R], bf16, tag="sqn")
                for m in range(4):
                    nc.vector.tensor_tensor(
                        out=sqn[:, m * R:(m + 1) * R],
                        in0=x1b[:, m * R:(m + 1) * R],
                        in1=x1b[:, m * R:(m + 1) * R], op=OP.mult)
                mun = tl.tile([128, R], f32, tag="mun")
                stn = tl.tile([128, R], f32, tag="stn")
                for src, dst, bb in ((x1b, mun, 0.0), (sqn, stn, LN_EPS)):
                    p = tp.tile([128, R], f32, tag="pt")
                    for k in range(4):
                        nc.tensor.matmul(p[:], ones_s[:],
                                         src[:, k * R:(k + 1) * R],
                                         start=(k == 0), stop=(k == 3))
                    nc.scalar.activation(dst[:], p[:], AF.Copy,
                                         bias=float(bb), scale=1.0 / DIM)
                mu2n = tl.tile([128, R], f32, tag="mu2n")
                nc.vector.tensor_tensor(out=mu2n[:], in0=mun[:], in1=mun[:],
                                        op=OP.mult)
                nc.vector.tensor_tensor(out=stn[:], in0=stn[:], in1=mu2n[:],
                                        op=OP.subtract)
                rsqrt_dve(tl, stn[:], "rsn")
                xlnn = tl.tile([128, 4 * R], bf16, tag="xlnn")
                for m in range(4):
                    sl = slice(m * R, (m + 1) * R)
                    nc.vector.tensor_tensor(out=xlnn[:, sl], in0=x1b[:, sl],
                                            in1=mun[:], op=OP.subtract)
                    nc.vector.tensor_tensor(out=xlnn[:, sl], in0=xlnn[:, sl],
                                            in1=stn[:], op=OP.mult)
                h2 = tl.tile([128, 16 * R], bf16, tag="h2")
                for m in range(16):
                    p = tp.tile([128, R], f32, tag="pt")
                    for k in range(4):
                        nc.tensor.matmul(
                            p[:],
                            w1n_s[:, k * 4 * DIM + m * 128:
                                  k * 4 * DIM + m * 128 + 128],
                            xlnn[:, k * R:(k + 1) * R],
                            start=(k == 0), stop=(k == 3))
                    nc.scalar.activation(h2[:, m * R:(m + 1) * R], p[:],
                                         AF.Gelu, bias=bia(B1NT, m))
                outs = tl.tile([128, 4 * R], f32, tag="outs")
                for m in range(4):
                    p = tp.tile([128, R], f32, tag="pt")
                    for k in range(16):
                        nc.tensor.matmul(
                            p[:],
                            w2n_s[:, k * DIM + m * 128:k * DIM + m * 128 + 128],
                            h2[:, k * R:(k + 1) * R],
                            start=(k == 0), stop=(k == 15))
                    sl = slice(m * R, (m + 1) * R)
                    nc.scalar.activation(p[:], p[:], AF.Identity,
                                         bias=bia(B2N, m))
                    nc.vector.tensor_tensor(out=outs[:, sl], in0=p[:],
                                            in1=x1f[:, sl], op=OP.add)
                    nc.sync.dma_start(out_t[m * 128:(m + 1) * 128, :],
                                      outs[:, sl])

    nc.compile()
    return nc


# =========================================================================
# host side
# =========================================================================
def _lhsT(w, d=None):
    """(K, M) f32 -> (128, kt*M) lhsT layout."""
    import ml_dtypes
    if d is None:
        d = ml_dtypes.bfloat16
    Kd, M = w.shape
    kt = Kd // 128
    return np.ascontiguousarray(
        w.reshape(kt, 128, M).transpose(1, 0, 2).reshape(128, kt * M)
    ).astype(d)


def _bias_cols(v):
    return np.ascontiguousarray(v.reshape(-1, 128).T).astype(np.float32)


def _prep(inputs):
    import ml_dtypes
    bf = ml_dtypes.bfloat16
    f8 = ml_dtypes.float8_e4m3fn
    f32 = np.float32

    node = np.asarray(inputs['node_repr'], f32).reshape(B * N, DIM)
    edge = np.asarray(inputs['edge_repr'], f32).reshape(B * N * K, PD)
    eidx = np.asarray(inputs['edge_index']).reshape(B, N, K)
    emask = np.asarray(inputs['edge_mask'], f32).reshape(B * N, K)
    mask_bw = np.asarray(inputs['mask_bw'], f32)
    if not np.all(mask_bw == 1.0):
        return None

    g = np.asarray(inputs['msg_ln_g'], f32)
    b = np.asarray(inputs['msg_ln_b'], f32)
    W1 = np.asarray(inputs['msg_W1'], f32)
    W1p = g[:, None] * W1
    b1t = np.asarray(inputs['msg_b1'], f32) + b @ W1
    gn = np.asarray(inputs['node_ln_g'], f32)
    bn = np.asarray(inputs['node_ln_b'], f32)
    W1n = np.asarray(inputs['node_W1'], f32)
    W1np = gn[:, None] * W1n
    b1nt = np.asarray(inputs['node_b1'], f32) + bn @ W1n

    sel = np.zeros((H, 8 * 128), f32)
    for tp in range(8):
        for hh in range(2):
            sel[2 * tp + hh,
                tp * 128 + 64 * hh:tp * 128 + 64 * hh + 64] = 1.0

    # node->edge indicator: ebk[n, s*512 + e] = 1 iff n == s*16 + e//32
    ebk = np.zeros((CNODE, CR), f32)
    for r in range(CR):
        ebk[r // K, r] = 1.0

    biases = np.zeros((128, 40), f32)
    biases[:, 0:8] = _bias_cols(b1t)
    biases[:, 8:10] = _bias_cols(np.asarray(inputs['msg_b2'], f32))
    biases[:, 10:12] = _bias_cols(np.asarray(inputs['b_gate'], f32))
    biases[:, 12:20] = _bias_cols(np.asarray(inputs['b_gat_gate'], f32))
    biases[:, 20:36] = _bias_cols(b1nt)
    biases[:, 36:40] = _bias_cols(np.asarray(inputs['node_b2'], f32))

    def ktile(xt):
        Kd, n = xt.shape
        kt = Kd // 128
        return np.ascontiguousarray(
            xt.reshape(kt, 128, n).transpose(1, 0, 2).reshape(128, kt * n))

    def krhs(w):
        Kd, M = w.shape
        kt = Kd // 128
        return np.ascontiguousarray(
            w.reshape(kt, 128, M).transpose(1, 0, 2).reshape(128, kt * M))

    rep = {
        'node_t': ktile(node.T).astype(bf),
        'w_src_r': krhs(np.asarray(inputs['W_node_src'], f32) * WS).astype(bf),
        'w_tgt_r': krhs(np.asarray(inputs['W_node_tgt'], f32) * WS).astype(bf),
        'w_edge_l': _lhsT(np.asarray(inputs['W_edge_msg'], f32) * WS, f8),
        'w1_l': _lhsT(W1p * WS, f8),
        'w2_l': _lhsT(np.asarray(inputs['msg_W2'], f32) * WS, f8),
        'wmsg_l': _lhsT(np.asarray(inputs['W_msg'], f32)),
        'wab_l': _lhsT(np.asarray(inputs['W_attn_bias'], f32) * LOG2E),
        'wv_l': _lhsT(np.asarray(inputs['W_gat_value'], f32)),
        'wgate_l': _lhsT(np.asarray(inputs['W_gate'], f32)),
        'wggate_l': _lhsT(np.asarray(inputs['W_gat_gate'], f32)),
        'wout_l': _lhsT(np.asarray(inputs['W_out'], f32)),
        'wgo_l': _lhsT(np.asarray(inputs['W_gat_out'], f32)),
        'w1n_l': _lhsT(W1np),
        'w2n_l': _lhsT(np.asarray(inputs['node_W2'], f32)),
        'sel_l': sel.astype(bf),
        'ebk_l': ebk.astype(bf),
        'biases': biases,
    }

    boff = (np.arange(B)[:, None, None] * N).astype(np.int32)
    gidx_full = (eidx.astype(np.int32) + boff).reshape(B * N, K)

    in_maps = []
    for c in range(NC):
        rs = slice(c * R, (c + 1) * R)
        esh = edge[c * RK:(c + 1) * RK, :]
        em = emask[rs, :].reshape(RK)
        gi = gidx_full[rs, :].reshape(RK)
        m = dict(rep)
        m['edge_t'] = np.ascontiguousarray(esh.T).astype(f8)
        m['gidx'] = np.ascontiguousarray(
            gi.reshape(NT, 128).T).astype(np.int32)
        emt = em.reshape(NT, 128).T
        m['emo'] = np.ascontiguousarray(
            (emt - 1.0) * MASK_OFF * LOG2E).astype(bf)
        m['emb'] = np.ascontiguousarray(emt).astype(bf)
        nsh = node[rs, :]
        m['node_sh_t'] = ktile(nsh.T).astype(bf)
        m['node_sh_f'] = ktile(nsh.T).astype(f32)
        in_maps.append(m)
    return in_maps


LAST_EXEC_NS = None
LAST_RESULTS = None


def _install_ntff_shim():
    """Provide antenv.axon_hooks (missing in this image) so trace=True works."""
    import sys
    import types
    import contextlib
    import ctypes
    try:
        from antenv.axon_hooks import get_axon_ntff_profile_hook  # noqa
        return
    except ImportError:
        pass
    so_path = "/opt/axon/libaxon_pjrt.so"
    try:
        lib = ctypes.CDLL(so_path)
    except OSError:
        lib = None
    hook = None
    if lib is not None and hasattr(lib, "axon_start_nrt_profile"):
        lib.axon_start_nrt_profile.argtypes = [
            ctypes.POINTER(ctypes.c_int64), ctypes.c_size_t]
        lib.axon_start_nrt_profile.restype = ctypes.c_int64
        lib.axon_stop_nrt_profile.argtypes = [ctypes.c_char_p]
        lib.axon_stop_nrt_profile.restype = ctypes.c_int64

        @contextlib.contextmanager
        def _hook(output_dir, device_ids):
            import jax
            jax.devices()
            if device_ids:
                ids = (ctypes.c_int64 * len(device_ids))(*device_ids)
                rc = lib.axon_start_nrt_profile(ids, len(device_ids))
            else:
                rc = lib.axon_start_nrt_profile(None, 0)
            if rc != 0:
                raise RuntimeError(f"axon_start_nrt_profile rc={rc}")
            try:
                yield
            finally:
                n = lib.axon_stop_nrt_profile(str(output_dir).encode())
                print(f"profile: {n} file(s) written to {output_dir}")

        hook = _hook
    mod = types.ModuleType("antenv.axon_hooks")
    mod.get_axon_ntff_profile_hook = lambda: hook
    mod.set_axon_ntff_profile_hook = lambda h: None
    sys.modules["antenv.axon_hooks"] = mod


def _run_device(in_maps, trace=False, tmpdir=None):
    global LAST_EXEC_NS, LAST_RESULTS
    if trace:
        try:
            _install_ntff_shim()
        except Exception:
            trace = False
    from concourse.bass_utils import run_bass_kernel_spmd
    if 'nc' not in _CACHE:
        _CACHE['nc'] = _build_nc()
    try:
        res = run_bass_kernel_spmd(
            _CACHE['nc'], in_maps, core_ids=list(range(NC)), trace=trace,
            tmpdir=tmpdir)
    except Exception:
        if not trace:
            raise
        res = run_bass_kernel_spmd(
            _CACHE['nc'], in_maps, core_ids=list(range(NC)), trace=False)
    if res.exec_time_ns:
        LAST_EXEC_NS = res.exec_time_ns
    LAST_RESULTS = res
    return res.results


def kernel(**inputs) -> np.ndarray:
    import os
    prep = _prep(inputs)
    if prep is None:
        raise RuntimeError("mask_bw != 1 unsupported")
    trace = os.environ.get("ATOM_TRACE", "0") == "1"
    outs = _run_device(prep, trace=trace)
    full = np.empty((B * N, DIM), np.float32)
    for c in range(NC):
        full[c * R:(c + 1) * R, :] = outs[c]['out_t'].T
    return full.reshape(B, N, DIM)


# revision 19
# speedup vs baseline: 1.6886x; 1.1495x over previous
"""AtomDecoderLayer (GNN message passing) on 8 trn2 NeuronCores via Bass/Tile.

Sharding: flattened (batch, node) rows 2*1024 = 2048 -> 8 shards of 256 nodes
(8192 edge-rows each). Weights replicated. The edge_index gather runs on
device: every core computes the full node_msg table (2048, 256) -> DRAM
scratch, then indirect-DMA-gathers its 8192 rows (one batched xbar transpose
per chunk moves them into feature-major layout).

Layout: activations are feature-on-partition ("T space"); every dense matmul
is lhsT=weight (K,M<=128 slices), rhs=activation^T, accumulating K tiles in
PSUM. fp32 accumulation. The edge-heavy matmuls (edge proj, msg MLP W1/W2)
run in fp8e4m3 with DoubleRow perf mode (2 k-tiles per matmul); their
weights are pre-scaled x64 on the host so 0.02-magnitude weights stay out of
fp8 subnormals, and the matching node src/tgt projections are scaled x64 in
bf16 (exact) so the shared msg0 PSUM is uniformly x64. LayerNorm is scale
invariant so the x64 washes out in xln; gelu/identity activations descale by
1/64 via the ACT scale input. LN gain/bias fold into the following weight on
the host; LN stats come from (1/256)-lhsT matmuls; rstd runs on DVE in bf16
via the int16 bit trick + 1 Newton step. The target-node projection is
injected into the msg0 PSUM by an extra matmul against a constant
node->edge indicator. The GAT attention einsum uses a block-diagonal rhs
trick: per 128-edge-row tile (4 nodes x 32 neighbors), lhsT = msg rows (row
space), rhs = A where A[:, 17g:17g+16] = exp(ab) of node g and
A[:, 17g+16] = edge mask; the matmul yields both the unnormalized
attention-weighted message sums (m) and the masked mean-pool sums, and a
ones-lhsT matmul over A yields softmax + pool denominators. W_attn_bias is
pre-scaled by log2(e) so exp becomes a bf16 2^y evaluated on DVE (round via
+192 magic, quadratic 2^f poly, exponent bits built in int16). og is a small
per-head projection of m (replaces the big msg @ W_gat_value matmul). The
attention tail of chunk c is pipelined into iteration c+1. zt uses the
Prelu activation (parametric_relu lives in every ACT table set, unlike
leaky_relu) so the chunk loop needs no ACT table switches at all.

Host does layout transforms / casts / index arithmetic only; all FLOPs run
on device. Self-contained: hardcodes b=2, n=1024, k=32, dim=512, pdim=256,
msg=256, H=16, D=64.
"""

import numpy as np

B, N, K = 2, 1024, 32
DIM, PD, MD = 512, 256, 256
H, DH = 16, 64
NC = 8
R = B * N // NC            # 256 nodes per core
RK = R * K                 # 8192 edge rows per core
NT = RK // 128             # 64 row-tiles per core
NCH = 8                    # chunks per core
CT = NT // NCH             # 8 row-tiles per chunk
CR = CT * 128              # 1024 edge rows per chunk
CNODE = CR // K            # 32 nodes per chunk
NS = CR // 512             # 2 512-col subtiles per chunk
LN_EPS = 1e-5
NEG = 0.01
MASK_OFF = 60.0
LOG2E = 1.4426950408889634
WS = 64.0                  # fp8 weight pre-scale (power of two)

_CACHE = {}


# =========================================================================
# device kernel (Bass IR)
# =========================================================================
def _build_nc(debug=False):
    import concourse.bacc as bacc
    import concourse.bass as bass
    import concourse.mybir as mybir
    import concourse.tile as tile

    dt = mybir.dt
    AF = mybir.ActivationFunctionType
    OP = mybir.AluOpType
    PM = mybir.MatmulPerfMode
    f32, bf16, f8 = dt.float32, dt.bfloat16, dt.float8e4

    nc = bacc.Bacc(None, target_bir_lowering=False)

    def din(name, shape, d=bf16):
        return nc.dram_tensor(name, shape, d, kind="ExternalInput")

    # per-core inputs
    edge_t = din("edge_t", [PD, RK], f8)             # edge_repr^T (fp8)
    gidx = din("gidx", [128, NT], dt.int32)          # gather row ids [p, t]
    emo = din("emo", [128, NT])                      # (em-1)*60*log2e bf16
    emb = din("emb", [128, NT])                      # em (bf16) [p, t]
    node_sh_t = din("node_sh_t", [128, 4 * R])       # node shard^T bf16
    node_sh_f = din("node_sh_f", [128, 4 * R], f32)
    # replicated inputs
    node_t = din("node_t", [128, 4 * B * N], f8)     # full node_repr^T fp8
    w_src_r = din("w_src_r", [128, 4 * MD], f8)      # rhs layout, x64
    w_edge_l = din("w_edge_l", [128, 2 * MD], f8)    # x64
    w_tgt_r = din("w_tgt_r", [128, 4 * MD])          # rhs layout, x64
    w1_l = din("w1_l", [128, 2 * 4 * MD], f8)        # x64, LN-g folded
    w2_l = din("w2_l", [128, 8 * MD], f8)            # x64
    wmsg_l = din("wmsg_l", [128, 2 * MD])
    wab_l = din("wab_l", [128, 2 * H])               # x log2e
    wv_l = din("wv_l", [128, 2 * H * DH])
    wgate_l = din("wgate_l", [128, 4 * MD])
    wggate_l = din("wggate_l", [128, 4 * H * DH])
    wout_l = din("wout_l", [128, 2 * DIM])
    wgo_l = din("wgo_l", [128, 8 * DIM])
    w1n_l = din("w1n_l", [128, 4 * 4 * DIM])
    w2n_l = din("w2n_l", [128, 16 * DIM])
    sel_l = din("sel_l", [H, 8 * 128])               # head-broadcast indicator
    ebk_l = din("ebk_l", [CNODE, CR])                # node->edge indicator
    biases = din("biases", [128, 40], f32)           # packed bias columns

    out_t = nc.dram_tensor("out_t", [DIM, R], f32, kind="ExternalOutput")

    dbg = {}
    if debug:
        for nm, shape, d in [
            ("d_gat", [128, 2 * CR], bf16), ("d_m0b", [128, 2 * CR], bf16),
            ("d_xln", [128, 2 * CR], f8), ("d_msg", [128, 2 * CR], bf16),
            ("d_abr", [128, CT * H], bf16), ("d_abm", [128, CT * H], bf16),
            ("d_A", [128, CT * 68], bf16), ("d_mrow", [128, 2 * CR], bf16),
            ("d_mT", [128, 2 * NT * 68], bf16), ("d_den", [1, NT * 68], f32),
            ("d_s16", [H, R], f32), ("d_g1", [128, 2 * R], f32),
            ("d_o", [128, 2 * R], bf16), ("d_og", [128, 8 * R], bf16),
            ("d_x1f", [128, 4 * R], f32), ("d_tgt", [CNODE, NCH * 2 * 128], bf16),
            ("d_rstd", [128, CR], bf16), ("d_h1", [128, 8 * 512], f8),
        ]:
            dbg[nm] = nc.dram_tensor(nm, shape, d, kind="ExternalOutput")

    # packed bias column offsets
    B1T, B2M, BGA, BGG, B1NT, B2N = 0, 8, 10, 12, 20, 36

    with tile.TileContext(nc) as tc:
        with (
            tc.tile_pool(name="per", bufs=1) as per,
            tc.tile_pool(name="wts", bufs=1) as wts,
            tc.tile_pool(name="dram", bufs=1, space="DRAM") as dpool,
        ):
            def load(t, shape, d=bf16, pool=wts):
                s = pool.tile(shape, d, tag=t.name)
                nc.sync.dma_start(s[:], t[:])
                return s

            node_sh_s = load(node_sh_t, [128, 4 * R], pool=per)
            node_shf_s = load(node_sh_f, [128, 4 * R], f32, pool=per)
            gidx_s = load(gidx, [128, NT], dt.int32, pool=per)
            emo_s = load(emo, [128, NT], pool=per)
            emb_s = load(emb, [128, NT], pool=per)
            bias_s = load(biases, [128, 40], f32, pool=per)

            def bia(base, m):
                return bias_s[:, base + m:base + m + 1]

            def rsqrt_dve(pool, ap, tag):
                """In-place y = 1/sqrt(ap) on DVE fp32 (bit trick + 1 Newton)."""
                shp = list(ap.shape)
                y0 = pool.tile(shp, f32, tag=tag + "_y0")
                t1 = pool.tile(shp, f32, tag=tag + "_t1")
                vi = ap.bitcast(dt.int32)
                y0i = y0[:].bitcast(dt.int32)
                nc.vector.tensor_scalar(
                    out=y0i, in0=vi, scalar1=1, scalar2=None,
                    op0=OP.arith_shift_right)
                nc.vector.tensor_scalar(
                    out=y0i, in0=y0i, scalar1=-1, scalar2=0x5f3759df,
                    op0=OP.mult, op1=OP.add)
                nc.vector.tensor_tensor(out=t1[:], in0=ap, in1=y0[:],
                                        op=OP.mult)
                nc.vector.tensor_tensor(out=t1[:], in0=t1[:], in1=y0[:],
                                        op=OP.mult)
                nc.vector.tensor_scalar(
                    out=t1[:], in0=t1[:], scalar1=-0.5, scalar2=1.5,
                    op0=OP.mult, op1=OP.add)
                nc.vector.tensor_tensor(out=ap, in0=y0[:], in1=t1[:],
                                        op=OP.mult)

            def rsqrt16(pool, ap, tag):
                """In-place y = 1/sqrt(ap) on DVE bf16 (int16 trick + Newton)."""
                shp = list(ap.shape)
                y0 = pool.tile(shp, bf16, tag=tag + "_y0")
                t1 = pool.tile(shp, bf16, tag=tag + "_t1")
                # int16 shifts are invalid ISA on DVE; the ALU computes in
                # fp32, so bits*-0.5 + magic == magic - (bits>>1) up to RNE.
                vi = ap.bitcast(dt.int16)
                y0i = y0[:].bitcast(dt.int16)
                nc.vector.tensor_scalar(
                    out=y0i, in0=vi, scalar1=-0.5, scalar2=0x5f37,
                    op0=OP.mult, op1=OP.add)
                nc.vector.tensor_tensor(out=t1[:], in0=ap, in1=y0[:],
                                        op=OP.mult)
                nc.vector.tensor_tensor(out=t1[:], in0=t1[:], in1=y0[:],
                                        op=OP.mult)
                nc.vector.tensor_scalar(
                    out=t1[:], in0=t1[:], scalar1=-0.5, scalar2=1.5,
                    op0=OP.mult, op1=OP.add)
                nc.vector.tensor_tensor(out=ap, in0=y0[:], in1=t1[:],
                                        op=OP.mult)

            ones_s = per.tile([128, 128], bf16, tag="ones")
            nc.vector.memset(ones_s[:], 1.0)
            ones_md = per.tile([128, 128], bf16, tag="ones_md")
            nc.vector.memset(ones_md[:], 1.0 / MD)

            g1 = per.tile([128, 2 * R], f32, tag="g1")
            g2 = per.tile([128, 8 * R], f32, tag="g2")

            table = dpool.tile([B * N, MD], bf16, tag="table")

            # persistent activations
            tgt32 = per.tile([CNODE, NCH * 2 * 128], bf16, tag="tgt32")
            mT_s = per.tile([128, 2 * NT * 68], bf16, tag="mT")   # einsum out
            den_s = per.tile([1, NT * 68], f32, tag="den")        # denominators
            den_v = den_s[:1, :].rearrange("o (c T g) -> o c T g", c=17, g=4)
            # denominators restreamed per chunk: [16, R] per-head softmax
            # dens + [1, R] pool dens (separate tiles: engine APs must
            # start at partition 0); col r = t*4 + g
            s16E = per.tile([H, R], f32, tag="s16E")
            sEE = per.tile([1, R], f32, tag="sEE")

            # ---------------- phase 0: node_msg table + tgt --------------
            with (
                tc.tile_pool(name="p0s", bufs=2) as p0s,
                tc.tile_pool(name="p0p", bufs=4, space="PSUM") as p0p,
            ):
                node_t_s = load(node_t, [128, 4 * B * N], f8, pool=p0s)
                w_src_s = load(w_src_r, [128, 4 * MD], f8, pool=p0s)
                w_tgt_s = load(w_tgt_r, [128, 4 * MD], pool=p0s)
                wgate_s = load(wgate_l, [128, 4 * MD], pool=p0s)
                wggate_s = load(wggate_l, [128, 4 * H * DH], pool=p0s)
                nt3 = node_t_s[:, :].rearrange("p (k x) -> p k x", k=4)
                ws3 = w_src_s[:, :].rearrange("p (k m) -> p k m", k=4)
                # 4 batched table writes (4 node-tiles each), fp8 DoubleRow
                for g in range(4):
                    sb = p0s.tile([128, 4 * MD], bf16, tag="tbev")
                    for i in range(4):
                        s = g * 4 + i
                        p = p0p.tile([128, MD], f32, tag="pt")
                        c0 = s * 128
                        for k in range(2):
                            nc.tensor.matmul(
                                p[:],
                                nt3[:, 2 * k:2 * k + 2, c0:c0 + 128],
                                ws3[:, 2 * k:2 * k + 2, :],
                                start=(k == 0), stop=(k == 1),
                                perf_mode=PM.DoubleRow)
                        nc.vector.tensor_copy(sb[:, i * MD:(i + 1) * MD], p[:])
                    nc.sync.dma_start(
                        table[g * 512:(g + 1) * 512, :].rearrange(
                            "(k p) m -> p k m", k=4),
                        sb[:, :].rearrange("p (k m) -> p k m", k=4))

                # tgt in row space, 32 nodes per chunk at partitions 0-31
                for c in range(NCH):
                    p = p0p.tile([CNODE, MD], f32, tag="pg")
                    for k in range(4):
                        nc.tensor.matmul(
                            p[:],
                            node_sh_s[:, k * R + c * CNODE:
                                      k * R + c * CNODE + CNODE],
                            w_tgt_s[:, k * MD:(k + 1) * MD],
                            start=(k == 0), stop=(k == 3))
                    nc.vector.tensor_copy(
                        tgt32[:, :].rearrange("p (c j e) -> p c (j e)", j=2,
                                              c=NCH)[:, c, :], p[:])
                if debug:
                    nc.sync.dma_start(dbg["d_tgt"][:], tgt32[:])

                # gates (raw; pool-denominator folded in at the tail)
                for m in range(2):
                    p = p0p.tile([128, R], f32, tag="pt")
                    for k in range(4):
                        nc.tensor.matmul(
                            p[:],
                            wgate_s[:, k * MD + m * 128:k * MD + m * 128 + 128],
                            node_sh_s[:, k * R:(k + 1) * R],
                            start=(k == 0), stop=(k == 3))
                    nc.scalar.activation(g1[:, m * R:(m + 1) * R], p[:],
                                         AF.Sigmoid, bias=bia(BGA, m))
                for m in range(8):
                    p = p0p.tile([128, R], f32, tag="pt")
                    for k in range(4):
                        nc.tensor.matmul(
                            p[:],
                            wggate_s[:, k * H * DH + m * 128:
                                     k * H * DH + m * 128 + 128],
                            node_sh_s[:, k * R:(k + 1) * R],
                            start=(k == 0), stop=(k == 3))
                    nc.scalar.activation(g2[:, m * R:(m + 1) * R], p[:],
                                         AF.Sigmoid, bias=bia(BGG, m))

            w_edge_s = load(w_edge_l, [128, 2 * MD], f8)
            w1_s = load(w1_l, [128, 2 * 4 * MD], f8)
            w2_s = load(w2_l, [128, 8 * MD], f8)
            wmsg_s = load(wmsg_l, [128, 2 * MD])
            wab_s = load(wab_l, [128, 2 * H])
            ebk_s = load(ebk_l, [CNODE, CR])
            wv_s = load(wv_l, [128, 2 * H * DH])
            sel_s = load(sel_l, [H, 8 * 128])
            wout_s = load(wout_l, [128, 2 * DIM])
            wgo_s = load(wgo_l, [128, 8 * DIM])
            w1n_s = load(w1n_l, [128, 4 * 4 * DIM])
            w2n_s = load(w2n_l, [128, 16 * DIM])

            we3 = w_edge_s[:, :].rearrange("p (k m) -> p k m", k=2)
            w13 = w1_s[:, :].rearrange("p (k m) -> p k m", k=2)
            w23 = w2_s[:, :].rearrange("p (k m) -> p k m", k=8)

            # ---------------- chunk loop (attention tail pipelined) ------
            with (
                tc.tile_pool(name="ch1", bufs=1) as ch1,
                tc.tile_pool(name="ch2", bufs=2) as ch2,
                tc.tile_pool(name="ch3", bufs=3) as ch3,
                tc.tile_pool(name="cpw", bufs=3, space="PSUM") as cpw,
                tc.tile_pool(name="cpm", bufs=2, space="PSUM") as cpm,
                tc.tile_pool(name="cpe", bufs=3, space="PSUM") as cpe,
            ):
                mT4 = mT_s[:, :].rearrange("p (j t c) -> p j t c", j=2, t=NT)

                def front(c):
                    c0 = c * CR
                    ech = ch2.tile([128, 2 * CR], f8, tag="ech")
                    for k in range(2):
                        nc.sync.dma_start(
                            ech[:, k * CR:(k + 1) * CR],
                            edge_t[k * 128:(k + 1) * 128, c0:c0 + CR])
                    ech3 = ech[:, :].rearrange("p (k x) -> p k x", k=2)

                    # gather (row space, ONE batched indirect DMA: the offset
                    # AP's p-major ravel pairs entry (p,t) with dst block
                    # grow[p, t*MD:(t+1)*MD]), then xbar to T space
                    grow = ch3.tile([128, CT * MD], bf16, tag="grow")
                    nc.gpsimd.indirect_dma_start(
                        out=grow[:], out_offset=None,
                        in_=table[:],
                        in_offset=bass.IndirectOffsetOnAxis(
                            ap=gidx_s[:, c * CT:(c + 1) * CT],
                            axis=0))
                    gat = ch2.tile([128, 2 * CR], bf16, tag="gat")
                    nc.sync.dma_start(
                        gat[:, :].rearrange("p (t j e) -> p (t j) e", j=2,
                                            e=128),
                        grow[:], transpose=True)
                    gat4 = gat[:, :].rearrange("p (t j e) -> p t j e",
                                               j=2, t=CT)
                    if debug and c == 0:
                        nc.sync.dma_start(dbg["d_gat"][:], gat[:])

                    # msg0 = edge@We (fp8 DR) + tgt-indicator-mm + gath (DVE)
                    m0b = ch2.tile([128, 2 * CR], bf16, tag="m0b")
                    for m in range(2):
                        for s in range(NS):
                            p = cpm.tile([128, 512], f32, tag="pm")
                            nc.tensor.matmul(
                                p[:],
                                we3[:, 0:2, m * 128:m * 128 + 128],
                                ech3[:, 0:2, s * 512:s * 512 + 512],
                                start=True, stop=False,
                                perf_mode=PM.DoubleRow)
                            nc.tensor.matmul(
                                p[:],
                                tgt32[:, (c * 2 + m) * 128:
                                      (c * 2 + m) * 128 + 128],
                                ebk_s[:, s * 512:s * 512 + 512],
                                start=False, stop=True)
                            p3t = p[:].rearrange("p (t e) -> p t e", e=128)
                            m3 = m0b[:, m * CR + s * 512:m * CR + s * 512
                                     + 512].rearrange("p (t e) -> p t e",
                                                      e=128)
                            nc.vector.tensor_tensor(
                                out=m3, in0=p3t,
                                in1=gat4[:, 4 * s:4 * s + 4, m, :],
                                op=OP.add)
                    if debug and c == 0:
                        nc.sync.dma_start(dbg["d_m0b"][:], m0b[:])

                    # LN stats (x64 units; LN is scale invariant, eps dropped)
                    sq = ch1.tile([128, 2 * CR], bf16, tag="sq")
                    for m in range(2):
                        nc.vector.tensor_tensor(
                            out=sq[:, m * CR:(m + 1) * CR],
                            in0=m0b[:, m * CR:(m + 1) * CR],
                            in1=m0b[:, m * CR:(m + 1) * CR], op=OP.mult)
                    mu = ch2.tile([128, CR], bf16, tag="mu")
                    st = ch2.tile([128, CR], bf16, tag="st")
                    for src, dst in ((m0b, mu), (sq, st)):
                        for s in range(NS):
                            p = cpm.tile([128, 512], f32, tag="pm")
                            for k in range(2):
                                nc.tensor.matmul(
                                    p[:], ones_md[:],
                                    src[:, k * CR + s * 512:
                                        k * CR + s * 512 + 512],
                                    start=(k == 0), stop=(k == 1))
                            nc.any.tensor_copy(
                                dst[:, s * 512:(s + 1) * 512], p[:])
                    mu2 = ch1.tile([128, CR], bf16, tag="mu2")
                    nc.vector.tensor_tensor(out=mu2[:], in0=mu[:], in1=mu[:],
                                            op=OP.mult)
                    nc.vector.tensor_tensor(out=st[:], in0=st[:], in1=mu2[:],
                                            op=OP.subtract)
                    rsqrt16(ch1, st[:], "rs")
                    if debug and c == 0:
                        nc.sync.dma_start(dbg["d_rstd"][:], st[:])
                    xln = ch2.tile([128, 2 * CR], f8, tag="xln")
                    xt = ch1.tile([128, CR], bf16, tag="xt")
                    for m in range(2):
                        sl = slice(m * CR, (m + 1) * CR)
                        nc.vector.tensor_tensor(out=xt[:], in0=m0b[:, sl],
                                                in1=mu[:], op=OP.subtract)
                        nc.vector.tensor_tensor(out=xln[:, sl], in0=xt[:],
                                                in1=st[:], op=OP.mult)
                    xln3 = xln[:, :].rearrange("p (k x) -> p k x", k=2)
                    if debug and c == 0:
                        nc.sync.dma_start(dbg["d_xln"][:], xln[:])

                    # msg MLP (fp8 DoubleRow, weights x64, descale in ACT)
                    msg = ch2.tile([128, 2 * CR], bf16, tag="msg")
                    for s in range(NS):
                        h1 = ch2.tile([128, 8 * 512], f8, tag="h1")
                        for m in range(8):
                            p = cpw.tile([128, 512], f32, tag="pw")
                            nc.tensor.matmul(
                                p[:],
                                w13[:, 0:2, m * 128:m * 128 + 128],
                                xln3[:, 0:2, s * 512:s * 512 + 512],
                                start=True, stop=True,
                                perf_mode=PM.DoubleRow)
                            nc.scalar.activation(
                                h1[:, m * 512:(m + 1) * 512], p[:], AF.Gelu,
                                bias=bia(B1T, m), scale=1.0 / WS)
                        if debug and c == 0 and s == 0:
                            nc.sync.dma_start(dbg["d_h1"][:], h1[:])
                        h13 = h1[:, :].rearrange("p (k x) -> p k x", k=8)
                        for m2 in range(2):
                            p = cpw.tile([128, 512], f32, tag="pw")
                            for t in range(4):
                                nc.tensor.matmul(
                                    p[:],
                                    w23[:, 2 * t:2 * t + 2,
                                        m2 * 128:m2 * 128 + 128],
                                    h13[:, 2 * t:2 * t + 2, :],
                                    start=(t == 0), stop=(t == 3),
                                    perf_mode=PM.DoubleRow)
                            nc.scalar.activation(
                                msg[:, m2 * CR + s * 512:
                                    m2 * CR + s * 512 + 512],
                                p[:], AF.Identity, bias=bia(B2M, m2),
                                scale=1.0 / WS)
                    if debug and c == 0:
                        nc.sync.dma_start(dbg["d_msg"][:], msg[:])

                    # z = prelu(msg @ Wmsg); ab = z @ (Wab * log2e)
                    zt = ch1.tile([128, 2 * CR], bf16, tag="zt")
                    for m in range(2):
                        for s in range(NS):
                            p = cpw.tile([128, 512], f32, tag="pw")
                            for k in range(2):
                                nc.tensor.matmul(
                                    p[:],
                                    wmsg_s[:, k * MD + m * 128:
                                           k * MD + m * 128 + 128],
                                    msg[:, k * CR + s * 512:
                                        k * CR + s * 512 + 512],
                                    start=(k == 0), stop=(k == 1))
                            nc.scalar.activation(
                                zt[:, m * CR + s * 512:m * CR + s * 512 + 512],
                                p[:], AF.Prelu, alpha=NEG)
                    abt = ch1.tile([H, CR], bf16, tag="abt")
                    for s in range(NS):
                        pfull = cpe.tile([128, 512], f32, tag="pback")
                        p = pfull[:H, :]
                        for k in range(2):
                            nc.tensor.matmul(
                                p[:], wab_s[:, k * H:(k + 1) * H],
                                zt[:, k * CR + s * 512:k * CR + s * 512 + 512],
                                start=(k == 0), stop=(k == 1))
                        nc.vector.tensor_copy(abt[:, s * 512:(s + 1) * 512],
                                              p[:])
                    abr = ch2.tile([128, CT * H], bf16, tag="abr")
                    nc.sync.dma_start(
                        abr[:, :].rearrange("p (t h) -> p t h", h=H),
                        abt[:], transpose=True)
                    if debug and c == 0:
                        nc.sync.dma_start(dbg["d_abr"][:], abr[:])

                    # msg -> row space (one xbar transpose)
                    mrow = ch2.tile([128, 2 * CR], bf16, tag="mrow")
                    nc.sync.dma_start(
                        mrow[:, :].rearrange("p (q e) -> p q e", e=128),
                        msg[:], transpose=True)
                    if debug and c == 0:
                        nc.sync.dma_start(dbg["d_mrow"][:], mrow[:])
                    mrow4 = mrow[:, :].rearrange("p (j t e) -> p j t e",
                                                 j=2, t=CT)
                    # abm = 2^(ab + emo) on DVE, all bf16:
                    # round via +192 magic; quad 2^f poly; exponent via int16
                    W = CT * H
                    MAGIC = 192.0
                    ey = ch1.tile([128, W], bf16, tag="ey")
                    ez = ch1.tile([128, W], bf16, tag="ez")
                    en = ch1.tile([128, W], bf16, tag="en")
                    ef = ch1.tile([128, W], bf16, tag="ef")
                    ep = ch1.tile([128, W], bf16, tag="ep")
                    e3 = emo_s[:, c * CT:(c + 1) * CT].to_broadcast(
                        [128, CT, H])
                    nc.vector.tensor_tensor(
                        out=ey[:].rearrange("p (t h) -> p t h", h=H),
                        in0=abr[:, :].rearrange("p (t h) -> p t h", h=H),
                        in1=e3, op=OP.add)
                    nc.vector.tensor_scalar_add(ez[:], ey[:], MAGIC)
                    nc.vector.tensor_scalar_sub(en[:], ez[:], MAGIC)
                    nc.vector.tensor_tensor(out=ef[:], in0=ey[:], in1=en[:],
                                            op=OP.subtract)
                    # p = 1 + f(ln2 + f c2)
                    nc.vector.tensor_scalar(
                        out=ep[:], in0=ef[:], scalar1=0.2416,
                        scalar2=0.6931472, op0=OP.mult, op1=OP.add)
                    nc.vector.tensor_tensor(out=ep[:], in0=ep[:], in1=ef[:],
                                            op=OP.mult)
                    nc.vector.tensor_scalar_add(ep[:], ep[:], 1.0)
                    # 2^n bits: (n+127)*128 as bf16 value -> int16 -> bitcast
                    nc.vector.tensor_scalar(
                        out=ez[:], in0=en[:], scalar1=128.0,
                        scalar2=16256.0, op0=OP.mult, op1=OP.add)
                    ei = ch1.tile([128, W], dt.int16, tag="ei")
                    nc.vector.tensor_copy(ei[:], ez[:])
                    abm = ch1.tile([128, CT * H], bf16, tag="abm")
                    abm3 = abm[:, :].rearrange("p (t h) -> p t h", h=H)
                    nc.vector.tensor_tensor(out=abm[:], in0=ep[:],
                                            in1=ei[:].bitcast(bf16),
                                            op=OP.mult)
                    if debug and c == 0:
                        nc.sync.dma_start(dbg["d_abm"][:], abm[:])

                    A = ch1.tile([128, CT * 68], bf16, tag="A")
                    nc.vector.memset(A[:], 0.0)
                    A3 = A[:, :].rearrange("p (t c) -> p t c", c=68)
                    em3 = emb_s[:, c * CT:(c + 1) * CT].to_broadcast(
                        [128, CT, 1])
                    for g in range(4):
                        rs = slice(32 * g, 32 * g + 32)
                        nc.vector.tensor_copy(A3[rs, :, 17 * g:17 * g + 16],
                                              abm3[rs, :, :])
                        nc.vector.tensor_copy(
                            A3[rs, :, 17 * g + 16:17 * g + 17], em3[rs, :, :])
                    if debug and c == 0:
                        nc.sync.dma_start(dbg["d_A"][:], A[:])

                    # einsum: batch 7 tiles per PSUM bank, 1 copy each m
                    for m in range(2):
                        pm_ = cpe.tile([128, 512], f32, tag="pback")
                        for t in range(7):
                            nc.tensor.matmul(
                                pm_[:, t * 68:(t + 1) * 68],
                                mrow4[:, m, t, :], A3[:, t, :],
                                start=True, stop=True)
                        nc.any.tensor_copy(
                            mT4[:, m, c * CT:c * CT + 7, :],
                            pm_[:, :476].rearrange("p (t c) -> p t c", c=68))
                    p7 = cpe.tile([128, 512], f32, tag="pback")
                    for m in range(2):
                        nc.tensor.matmul(
                            p7[:, m * 68:m * 68 + 68],
                            mrow4[:, m, 7, :], A3[:, 7, :],
                            start=True, stop=True)
                    for m in range(2):
                        nc.any.tensor_copy(mT4[:, m, c * CT + 7, :],
                                           p7[:, m * 68:m * 68 + 68])
                    for t0, nt_ in ((0, 7), (7, 1)):
                        w = nt_ * 68
                        p = cpe.tile([128, 512], f32, tag="pback")
                        nc.tensor.matmul(p[:, :w], ones_s[:],
                                         A[:, t0 * 68:t0 * 68 + w],
                                         start=True, stop=True)
                        nc.scalar.copy(
                            den_v[:, :, c * CT + t0:c * CT + t0 + nt_, :]
                            .rearrange("o c T g -> o T g c"),
                            p[:1, :w].rearrange("o (t g c) -> o t g c",
                                                g=4, c=17))
                    # restream this chunk's denominators so the tail doesn't
                    # wait on one big SBUF->SBUF DMA (1-partition source
                    # fans out to 16 dest partitions in AP order)
                    nc.sync.dma_start(
                        s16E[:, c * CT * 4:(c + 1) * CT * 4],
                        den_v[:, 0:H, c * CT:(c + 1) * CT, :])
                    nc.sync.dma_start(
                        sEE[:, c * CT * 4:(c + 1) * CT * 4],
                        den_v[:, H:H + 1, c * CT:(c + 1) * CT, :])

                for c in range(NCH):
                    front(c)

            # ---------------- tail ---------------------------------------
            with (
                tc.tile_pool(name="tl", bufs=1) as tl,
                tc.tile_pool(name="tp", bufs=4, space="PSUM") as tp,
                tc.tile_pool(name="tp2", bufs=2, space="PSUM") as tp2,
            ):
                if debug:
                    nc.sync.dma_start(dbg["d_mT"][:], mT_s[:])
                    nc.sync.dma_start(dbg["d_den"][:], den_s[:])
                # s16(h, node) = 1/softmax_den; sE(node) = 1/(pool_den+1e-6)
                s16 = s16E[:, :]
                rsqrt_dve(tl, s16, "rs16")
                nc.vector.tensor_tensor(out=s16, in0=s16, in1=s16,
                                        op=OP.mult)
                if debug:
                    nc.sync.dma_start(dbg["d_s16"][:], s16)
                s16b = tl.tile([H, R], bf16, tag="s16b")
                nc.vector.tensor_copy(s16b[:], s16)
                sE = sEE[:, :]
                nc.vector.tensor_scalar_add(sE, sE, 1e-6)
                rsqrt_dve(tl, sE, "rsE")
                nc.vector.tensor_tensor(out=sE, in0=sE, in1=sE,
                                        op=OP.mult)
                sEb = tl.tile([1, R], bf16, tag="sEb")
                nc.vector.tensor_copy(sEb[:], sE)
                sE_ps = tp2.tile([128, R], f32, tag="psE")
                nc.tensor.matmul(sE_ps[:], ones_s[:1, :], sEb[:],
                                 start=True, stop=True)

                # fold pool denominator into g1
                for m in range(2):
                    nc.vector.tensor_tensor(out=g1[:, m * R:(m + 1) * R],
                                            in0=g1[:, m * R:(m + 1) * R],
                                            in1=sE_ps[:], op=OP.mult)
                if debug:
                    nc.sync.dma_start(dbg["d_g1"][:], g1[:])

                mT5 = mT_s[:, :].rearrange("p (j t g c) -> p j t g c",
                                           j=2, t=NT, g=4)
                # o^T gated
                o_s = tl.tile([128, 2 * R], bf16, tag="o")
                for m in range(2):
                    nc.vector.tensor_tensor(
                        out=o_s[:, m * R:(m + 1) * R].rearrange(
                            "p (t g) -> p t g", g=4),
                        in0=mT5[:, m, :, :, 16],
                        in1=g1[:, m * R:(m + 1) * R].rearrange(
                            "p (t g) -> p t g", g=4),
                        op=OP.mult)
                if debug:
                    nc.sync.dma_start(dbg["d_o"][:], o_s[:])

                # og: per-head projection of m, then softmax scale and gate
                og_s = tl.tile([128, 8 * R], bf16, tag="og")
                for tpi in range(8):
                    p = tp.tile([128, R], f32, tag="pt")
                    for hh in range(2):
                        h = 2 * tpi + hh
                        for k in range(2):
                            nc.tensor.matmul(
                                p[64 * hh:64 * hh + 64, :],
                                wv_s[:, k * H * DH + h * DH:
                                     k * H * DH + h * DH + DH],
                                mT5[:, k, :, :, h].rearrange(
                                    "p t g -> p (t g)"),
                                start=(k == 0), stop=(k == 1))
                    sp = tp2.tile([128, R], f32, tag="ps16")
                    nc.tensor.matmul(sp[:], sel_s[:, tpi * 128:(tpi + 1) * 128],
                                     s16b[:], start=True, stop=True)
                    sg = tl.tile([128, R], f32, tag="sg")
                    nc.vector.tensor_tensor(out=sg[:], in0=sp[:],
                                            in1=g2[:, tpi * R:(tpi + 1) * R],
                                            op=OP.mult)
                    nc.vector.tensor_tensor(
                        out=og_s[:, tpi * R:(tpi + 1) * R], in0=p[:],
                        in1=sg[:], op=OP.mult)
                if debug:
                    nc.sync.dma_start(dbg["d_og"][:], og_s[:])

                # dh = W_out.T @ o + W_go.T @ og;  x1 = node + dh
                x1f = tl.tile([128, 4 * R], f32, tag="x1f")
                x1b = tl.tile([128, 4 * R], bf16, tag="x1b")
                for m in range(4):
                    p = tp.tile([128, R], f32, tag="pt")
                    for k in range(2):
                        nc.tensor.matmul(
                            p[:],
                            wout_s[:, k * DIM + m * 128:k * DIM + m * 128 + 128],
                            o_s[:, k * R:(k + 1) * R],
                            start=(k == 0), stop=False)
                    for k in range(8):
                        nc.tensor.matmul(
                            p[:],
                            wgo_s[:, k * DIM + m * 128:k * DIM + m * 128 + 128],
                            og_s[:, k * R:(k + 1) * R],
                            start=False, stop=(k == 7))
                    sl = slice(m * R, (m + 1) * R)
                    nc.vector.tensor_tensor(out=x1f[:, sl], in0=p[:],
                                            in1=node_shf_s[:, sl], op=OP.add)
                    nc.vector.tensor_copy(x1b[:, sl], x1f[:, sl])
                if debug:
                    nc.sync.dma_start(dbg["d_x1f"][:], x1f[:])

                # node MLP with LN folded into W1n (bf16)
                sqn = tl.tile([128, 4 * R], bf16, tag="sqn")
                for m in range(4):
                    nc.vector.tensor_tensor(
                        out=sqn[:, m * R:(m + 1) * R],
                        in0=x1b[:, m * R:(m + 1) * R],
                        in1=x1b[:, m * R:(m + 1) * R], op=OP.mult)
                mun = tl.tile([128, R], f32, tag="mun")
                stn = tl.tile([128, R], f32, tag="stn")
                for src, dst, bb in ((x1b, mun, 0.0), (sqn, stn, LN_EPS)):
                    p = tp.tile([128, R], f32, tag="pt")
                    for k in range(4):
                        nc.tensor.matmul(p[:], ones_s[:],
                                         src[:, k * R:(k + 1) * R],
                                         start=(k == 0), stop=(k == 3))
                    nc.scalar.activation(dst[:], p[:], AF.Copy,
                                         bias=float(bb), scale=1.0 / DIM)
                mu2n = tl.tile([128, R], f32, tag="mu2n")
                nc.vector.tensor_tensor(out=mu2n[:], in0=mun[:], in1=mun[:],
                                        op=OP.mult)
                nc.vector.tensor_tensor(out=stn[:], in0=stn[:], in1=mu2n[:],
                                        op=OP.subtract)
                rsqrt_dve(tl, stn[:], "rsn")
                xlnn = tl.tile([128, 4 * R], bf16, tag="xlnn")
                for m in range(4):
                    sl = slice(m * R, (m + 1) * R)
                    nc.vector.tensor_tensor(out=xlnn[:, sl], in0=x1b[:, sl],
                                            in1=mun[:], op=OP.subtract)
                    nc.vector.tensor_tensor(out=xlnn[:, sl], in0=xlnn[:, sl],
                                            in1=stn[:], op=OP.mult)
                h2 = tl.tile([128, 16 * R], bf16, tag="h2")
                for m in range(16):
                    p = tp.tile([128, R], f32, tag="pt")
                    for k in range(4):
                        nc.tensor.matmul(
                            p[:],
                            w1n_s[:, k * 4 * DIM + m * 128:
                                  k * 4 * DIM + m * 128 + 128],
                            xlnn[:, k * R:(k + 1) * R],
                            start=(k == 0), stop=(k == 3))
                    nc.scalar.activation(h2[:, m * R:(m + 1) * R], p[:],
                                         AF.Gelu, bias=bia(B1NT, m))
                outs = tl.tile([128, 4 * R], f32, tag="outs")
                for m in range(4):
                    p = tp.tile([128, R], f32, tag="pt")
                    for k in range(16):
                        nc.tensor.matmul(
                            p[:],
                            w2n_s[:, k * DIM + m * 128:k * DIM + m * 128 + 128],
                            h2[:, k * R:(k + 1) * R],
                            start=(k == 0), stop=(k == 15))
                    sl = slice(m * R, (m + 1) * R)
                    nc.scalar.activation(p[:], p[:], AF.Identity,
                                         bias=bia(B2N, m))
                    nc.vector.tensor_tensor(out=outs[:, sl], in0=p[:],
                                            in1=x1f[:, sl], op=OP.add)
                    nc.sync.dma_start(out_t[m * 128:(m + 1) * 128, :],
                                      outs[:, sl])

    nc.compile()
    return nc


# =========================================================================
# host side
# =========================================================================
def _lhsT(w, d=None):
    """(K, M) f32 -> (128, kt*M) lhsT layout."""
    import ml_dtypes
    if d is None:
        d = ml_dtypes.bfloat16
    Kd, M = w.shape
    kt = Kd // 128
    return np.ascontiguousarray(
        w.reshape(kt, 128, M).transpose(1, 0, 2).reshape(128, kt * M)
    ).astype(d)


def _bias_cols(v):
    return np.ascontiguousarray(v.reshape(-1, 128).T).astype(np.float32)


def _prep(inputs):
    import ml_dtypes
    bf = ml_dtypes.bfloat16
    f8 = ml_dtypes.float8_e4m3fn
    f32 = np.float32

    node = np.asarray(inputs['node_repr'], f32).reshape(B * N, DIM)
    edge = np.asarray(inputs['edge_repr'], f32).reshape(B * N * K, PD)
    eidx = np.asarray(inputs['edge_index']).reshape(B, N, K)
    emask = np.asarray(inputs['edge_mask'], f32).reshape(B * N, K)
    mask_bw = np.asarray(inputs['mask_bw'], f32)
    if not np.all(mask_bw == 1.0):
        return None

    g = np.asarray(inputs['msg_ln_g'], f32)
    b = np.asarray(inputs['msg_ln_b'], f32)
    W1 = np.asarray(inputs['msg_W1'], f32)
    W1p = g[:, None] * W1
    b1t = np.asarray(inputs['msg_b1'], f32) + b @ W1
    gn = np.asarray(inputs['node_ln_g'], f32)
    bn = np.asarray(inputs['node_ln_b'], f32)
    W1n = np.asarray(inputs['node_W1'], f32)
    W1np = gn[:, None] * W1n
    b1nt = np.asarray(inputs['node_b1'], f32) + bn @ W1n

    sel = np.zeros((H, 8 * 128), f32)
    for tp in range(8):
        for hh in range(2):
            sel[2 * tp + hh,
                tp * 128 + 64 * hh:tp * 128 + 64 * hh + 64] = 1.0

    # node->edge indicator: ebk[n, s*512 + e] = 1 iff n == s*16 + e//32
    ebk = np.zeros((CNODE, CR), f32)
    for r in range(CR):
        ebk[r // K, r] = 1.0

    biases = np.zeros((128, 40), f32)
    biases[:, 0:8] = _bias_cols(b1t)
    biases[:, 8:10] = _bias_cols(np.asarray(inputs['msg_b2'], f32))
    biases[:, 10:12] = _bias_cols(np.asarray(inputs['b_gate'], f32))
    biases[:, 12:20] = _bias_cols(np.asarray(inputs['b_gat_gate'], f32))
    biases[:, 20:36] = _bias_cols(b1nt)
    biases[:, 36:40] = _bias_cols(np.asarray(inputs['node_b2'], f32))

    def ktile(xt):
        Kd, n = xt.shape
        kt = Kd // 128
        return np.ascontiguousarray(
            xt.reshape(kt, 128, n).transpose(1, 0, 2).reshape(128, kt * n))

    def krhs(w):
        Kd, M = w.shape
        kt = Kd // 128
        return np.ascontiguousarray(
            w.reshape(kt, 128, M).transpose(1, 0, 2).reshape(128, kt * M))

    rep = {
        'node_t': ktile(node.T).astype(f8),
        'w_src_r': krhs(np.asarray(inputs['W_node_src'], f32) * WS).astype(f8),
        'w_tgt_r': krhs(np.asarray(inputs['W_node_tgt'], f32) * WS).astype(bf),
        'w_edge_l': _lhsT(np.asarray(inputs['W_edge_msg'], f32) * WS, f8),
        'w1_l': _lhsT(W1p * WS, f8),
        'w2_l': _lhsT(np.asarray(inputs['msg_W2'], f32) * WS, f8),
        'wmsg_l': _lhsT(np.asarray(inputs['W_msg'], f32)),
        'wab_l': _lhsT(np.asarray(inputs['W_attn_bias'], f32) * LOG2E),
        'wv_l': _lhsT(np.asarray(inputs['W_gat_value'], f32)),
        'wgate_l': _lhsT(np.asarray(inputs['W_gate'], f32)),
        'wggate_l': _lhsT(np.asarray(inputs['W_gat_gate'], f32)),
        'wout_l': _lhsT(np.asarray(inputs['W_out'], f32)),
        'wgo_l': _lhsT(np.asarray(inputs['W_gat_out'], f32)),
        'w1n_l': _lhsT(W1np),
        'w2n_l': _lhsT(np.asarray(inputs['node_W2'], f32)),
        'sel_l': sel.astype(bf),
        'ebk_l': ebk.astype(bf),
        'biases': biases,
    }

    boff = (np.arange(B)[:, None, None] * N).astype(np.int32)
    gidx_full = (eidx.astype(np.int32) + boff).reshape(B * N, K)

    in_maps = []
    for c in range(NC):
        rs = slice(c * R, (c + 1) * R)
        esh = edge[c * RK:(c + 1) * RK, :]
        em = emask[rs, :].reshape(RK)
        gi = gidx_full[rs, :].reshape(RK)
        m = dict(rep)
        m['edge_t'] = np.ascontiguousarray(esh.T).astype(f8)
        m['gidx'] = np.ascontiguousarray(
            gi.reshape(NT, 128).T).astype(np.int32)
        emt = em.reshape(NT, 128).T
        m['emo'] = np.ascontiguousarray(
            (emt - 1.0) * MASK_OFF * LOG2E).astype(bf)
        m['emb'] = np.ascontiguousarray(emt).astype(bf)
        nsh = node[rs, :]
        m['node_sh_t'] = ktile(nsh.T).astype(bf)
        m['node_sh_f'] = ktile(nsh.T).astype(f32)
        in_maps.append(m)
    return in_maps


LAST_EXEC_NS = None
LAST_RESULTS = None


def _install_ntff_shim():
    """Provide antenv.axon_hooks (missing in this image) so trace=True works."""
    import sys
    import types
    import contextlib
    import ctypes
    try:
        from antenv.axon_hooks import get_axon_ntff_profile_hook  # noqa
        return
    except ImportError:
        pass
    so_path = "/opt/axon/libaxon_pjrt.so"
    try:
        lib = ctypes.CDLL(so_path)
    except OSError:
        lib = None
    hook = None
    if lib is not None and hasattr(lib, "axon_start_nrt_profile"):
        lib.axon_start_nrt_profile.argtypes = [
            ctypes.POINTER(ctypes.c_int64), ctypes.c_size_t]
        lib.axon_start_nrt_profile.restype = ctypes.c_int64
        lib.axon_stop_nrt_profile.argtypes = [ctypes.c_char_p]
        lib.axon_stop_nrt_profile.restype = ctypes.c_int64

        @contextlib.contextmanager
        def _hook(output_dir, device_ids):
            import jax
            jax.devices()
            if device_ids:
                ids = (ctypes.c_int64 * len(device_ids))(*device_ids)
                rc = lib.axon_start_nrt_profile(ids, len(device_ids))
            else:
                rc = lib.axon_start_nrt_profile(None, 0)
            if rc != 0:
                raise RuntimeError(f"axon_start_nrt_profile rc={rc}")
            try:
                yield
            finally:
                n = lib.axon_stop_nrt_profile(str(output_dir).encode())
                print(f"profile: {n} file(s) written to {output_dir}")

        hook = _hook
    mod = types.ModuleType("antenv.axon_hooks")
    mod.get_axon_ntff_profile_hook = lambda: hook
    mod.set_axon_ntff_profile_hook = lambda h: None
    sys.modules["antenv.axon_hooks"] = mod


def _run_device(in_maps, trace=False, tmpdir=None):
    global LAST_EXEC_NS, LAST_RESULTS
    if trace:
        try:
            _install_ntff_shim()
        except Exception:
            trace = False
    from concourse.bass_utils import run_bass_kernel_spmd
    if 'nc' not in _CACHE:
        _CACHE['nc'] = _build_nc()
    try:
        res = run_bass_kernel_spmd(
            _CACHE['nc'], in_maps, core_ids=list(range(NC)), trace=trace,
            tmpdir=tmpdir)
    except Exception:
        if not trace:
            raise
        res = run_bass_kernel_spmd(
            _CACHE['nc'], in_maps, core_ids=list(range(NC)), trace=False)
    if res.exec_time_ns:
        LAST_EXEC_NS = res.exec_time_ns
    LAST_RESULTS = res
    return res.results


def kernel(**inputs) -> np.ndarray:
    import os
    prep = _prep(inputs)
    if prep is None:
        raise RuntimeError("mask_bw != 1 unsupported")
    trace = os.environ.get("ATOM_TRACE", "0") == "1"
    outs = _run_device(prep, trace=trace)
    full = np.empty((B * N, DIM), np.float32)
    for c in range(NC):
        full[c * R:(c + 1) * R, :] = outs[c]['out_t'].T
    return full.reshape(B, N, DIM)
